# revision 51
# baseline (speedup 1.0000x reference)
"""Trainium2 Bass kernel for nn_BackProjLayer.

Math: the reference computes, per sample n,
    eigh(S) -> (lam, V);  G = V @ diag(sqrt(max(lam,0)));  y = D^H G
    out[n,p] = sum_d |y[p,d]|^2 - tau[p] = [D^H S_plus D]_pp - tau[p]
Since S = A A^H / Nch is Hermitian PSD by construction, S_plus == S up to
float32 eigensolver noise, so no eigendecomposition is needed:
    out[n,p] = Re(d_p^H S[n] d_p) - tau[p]
With S = Sr + i Si (Sr sym, Si antisym) and d = dr + i di this is a real
bilinear form; by Hermitian symmetry it reduces to 16 features per sample
(4 diag Sr, 6 offdiag Sr doubled, 6 offdiag Si doubled):
    out = X16.T @ W16 - tau     X16 (16, N), W16 (16, 242)

Device kernel (default layout "H2", single fp16 plane):
  - fp16 inputs/outputs: with the 2e-2 rel-err budget, one fp16 plane of
    X16/W16 plus fp16 output rounding gives 3.4e-4 Frobenius rel err;
    tau enters exactly via two const-one rows against (-tau) hi/lo fp16
    planes (K=18). Output ships as fp16 and the host upcasts, halving
    HBM write traffic vs fp32.
  - pixels on PSUM partitions, samples on the moving dim. The PE here is
    clock-capped at 1.2 GHz and K=18 uses 18 of 128 array rows, so the
    kernel row-tiles: pixel chunk A (px 0-127, array rows 0-17,
    tile_position (0,0)) and chunk B (px 128-241 zero-padded to 128
    stationary cols, rows 32-49, tile_position (32,0)) run as two
    CONCURRENT matmuls per 512-sample slice - one pass over samples
    instead of two. X is host-replicated at SBUF partitions 0-17/32-49.
  - both matmuls of a slice write one shared 2-bank PSUM tile; a single
    plain fp16-cast copy (alternating vector/scalar) evacuates A+B
    together (B's junk pixel rows ride free - partitions are parallel
    lanes); 8 grouped ~0.5MB contiguous output DMAs alternate the two
    HWDGE queues (sync/scalar); host strips junk rows and transposes.
  - lightweight Tile exit (tail="no_reset"): the explicit sync.drain()
    already guarantees DMA completion, so the gpsimd dma_reset is
    skipped and only the semaphore clears run.

Sharding: pure data parallel over N across 8 cores (8192 samples/core);
host packs per-core inputs, device returns fp16 blocks per core, host
unscrambles, concatenates and upcasts to fp32.
"""

import sys

for _p in ("/opt/trn_rl_repo", "/root/.axon_site/_ro/trn_rl_repo"):
    if _p not in sys.path:
        sys.path.insert(0, _p)

import numpy as np

N_SAMPLES = 65536
N_CH = 4
N_PX = 242
N_CORES = 8
N_LOC = N_SAMPLES // N_CORES  # 8192

K_FEAT = 2 * N_CH * N_CH + 1  # 33

TILE = 128
N_TILES = N_LOC // TILE  # 64

DEFAULT_CFG = dict(
    layout="H2",     # H2: fp16 row-tiled, single-copy evac; H: fp16 row-tiled;
                     # G: fp16 pixels-on-partitions; F: fp16 samples-on-partitions;
                     # A: bf16split samples-on-partitions; B: bf16split pixels-on-partitions
    bf16split=True,  # exact-fp32 via K-stacked bf16 3-way split (K=99)
    f32r=True,       # (non-split A only) float32r matmul
    pack=2,          # matmul outputs per PSUM tile / per copy instruction
    psum_bufs=4,     # PSUM pool slots (H2: 4 x 2-bank slots = full PSUM)
    groups=8,        # F/A: number of output DMAs
    dma_engines=("sync", "scalar"),  # round-robin for output DMAs
    copy_pattern="vs",  # per-copy engine cycle: s=scalar, v=vector
    x_chunks=4,      # input DMA chunks
    h_chunks=(512, 3584, 4096),  # H/H2: per-row-group X chunk widths
    slice_w=512,     # B/G: samples per matmul (moving dim)
    group_w=1024,    # B/G/H2: samples per output DMA group
    warm_mms=0,      # G: dummy matmuls during the input-DMA wait (PE is
                     # clock-capped at 1.2GHz on this part, so HAM warmup
                     # only delays real work)
    warm_tables=True,  # G: tiny scalar op to pull ACT_TABLE_LOAD early
    tau_bias=True,   # G: K=16, tau as per-partition bias in the copies
    tail="no_reset", # H2 default; see _tc_class
    ldw_dedup=True,  # H2: shrink redundant LDWEIGHTS to 1 column
    h_groups=None,   # H2: output DMA group widths; None = uniform group_w.
                     # Uneven tails measured a wash: the extra issue slots
                     # land between the final copies and eat the drain gain.
    dma_warm=False,  # H2: dummy first DMA did NOT absorb the issue penalty,
                     # it just delayed chunk0 (~-0.7us)
    in_eng="sync",   # engine issuing input DMAs
    in_eng2="sync",  # H2: engine for the second first-chunk DMA (scalar's
                     # first D2D costs ~1.6us, so parallel issue gains nothing)
    linear_out=True, # B: write output groups as contiguous HBM blocks
    pmajor=True,     # A: partition-major sample mapping (contiguous 7.7KB
                     # HBM runs per partition in output DMAs)
)

# B layout constants
PX_CHUNKS = (128, 114)  # pixels per chunk (DMA uses 8 partitions/SDMA engine,
                        # so a 128-partition transfer engages all 16 engines)
W_PAD = 242          # leading cols of xw holding the stacked W planes
IDX_DIAG = [0, 5, 10, 15]       # S[c,c] positions in the c*4+c' flattening
IDX_OFF = [1, 2, 3, 6, 7, 11]   # S[c,c'] c<c' positions

_BUILT = {}


def _tc_class(base, cfg):
    """Optionally lighten the Tile kernel-tail: keep the drain (output DMA
    completion) and the semaphore clears (needed for NEFF re-execution),
    but trim barrier work per cfg['tail'] mode."""
    mode = cfg.get("tail", "full")
    if mode == "full":
        return base

    from concourse.vector_clock import ScopedClock

    class _TC(base):
        def _drain_and_barrier(self, tick_clock, wait_clock):
            nc = self.nc
            drain_inst = nc.sync.drain()
            wait_clock.add_sem_waits(
                drain_inst.ins, ScopedClock({None: tick_clock.global_clock})
            )
            if mode in ("sem_only", "no_reset"):
                nc.all_engine_barrier(sem_only=True)
            else:
                nc.all_engine_barrier()
            popped = nc._tile_sem_poison_stack.pop()
            assert popped is self._sem_poison
            sems = list(self.sems.allocated().values())
            if mode == "no_reset":
                # the sync.drain() above already guarantees every DMA
                # completed, so the gpsimd dma_reset drain is redundant;
                # just zero the sems for NEFF re-execution
                from concourse.bass import compact_to_ranges

                sem_nums = [s.num if hasattr(s, "num") else s for s in sems]
                for sem_range in compact_to_ranges(sem_nums):
                    nc.gpsimd.sem_clear(sem_range)
                nc._state.prepend_free_semaphores(sem_nums)
                for poison_set in nc._tile_sem_poison_stack:
                    poison_set.update(sem_nums)
            else:
                nc.clear_and_free_semaphores(sems)
            if mode not in ("no2nd", "sem_only", "no_reset"):
                nc.all_engine_barrier()

    return _TC


def _build_nc(cfg):
    import concourse.mybir as mybir
    from concourse import bacc
    from concourse.tile import TileContext

    f32 = mybir.dt.float32
    f32r = mybir.dt.float32r
    bf16 = mybir.dt.bfloat16

    if cfg.get("bf16split"):
        # exact-fp32 bf16 3-way split (see _build_nc_b docstring), with two
        # extra const-one rows pairing against the -tau bf16 planes: K=99
        pxp = N_PX
        in_dt = bf16
        kf = 99
    else:
        pxp = 256 if cfg["f32r"] else N_PX
        in_dt = f32r if cfg["f32r"] else f32
        kf = K_FEAT
    ps_stride = 256
    pack = cfg["pack"]
    groups = cfg["groups"]
    tiles_per_group = N_TILES // groups
    assert tiles_per_group % pack == 0 or pack % tiles_per_group == 0

    # Bacc (not plain Bass): its compile() lowers multi-wait sync_infos into
    # chained EventSemaphores (TRN2 allows 1 wait/instruction).
    nc = bacc.Bacc("TRN2", target_bir_lowering=False, debug=False)
    TileContext = _tc_class(TileContext, cfg)
    xTw = nc.declare_dram_parameter("xTw", [kf, pxp + N_LOC], in_dt, isOutput=False)
    out = nc.declare_dram_parameter("out", [N_LOC, N_PX], f32, isOutput=True)

    if cfg.get("pmajor"):
        # partition-major sample mapping: tile t, partition p <-> sample
        # n = p*64 + t. Each partition's 8-tile group lands in 8
        # CONSECUTIVE output rows -> 7744B contiguous HBM runs per
        # partition (vs 968B strided), much better SDMA descriptor
        # efficiency. Host permutes the input columns to match.
        out_g = out.rearrange(
            "(p g j) c -> g p (j c)", p=TILE, g=groups, j=tiles_per_group
        )
    else:
        out_g = out.rearrange("(g j p) c -> g p j c", p=TILE, j=tiles_per_group)
    x_chunk = N_LOC // cfg["x_chunks"]

    with TileContext(nc) as tc:
        with (
            tc.tile_pool(name="xin", bufs=1) as xpool,
            tc.tile_pool(name="ps", bufs=cfg["psum_bufs"], space="PSUM") as pspool,
            tc.tile_pool(name="ob", bufs=1) as opool,
        ):
            if cfg.get("sep_in"):
                in_eng = getattr(nc, cfg["in_eng"])
                wt_tile = xpool.tile([kf, pxp], in_dt, tag="w")
                in_eng.dma_start(wt_tile[:], xTw[:, :pxp])
                wt = wt_tile[:]
                xts = []
                for ci in range(cfg["x_chunks"]):
                    xt = xpool.tile([kf, x_chunk], in_dt, tag=f"x{ci}")
                    lo = pxp + ci * x_chunk
                    in_eng.dma_start(xt[:], xTw[:, lo : lo + x_chunk])
                    xts.append(xt)

                def lhs_ap(t):
                    ci, off = divmod(t * TILE, x_chunk)
                    return xts[ci][:, off : off + TILE]
            else:
                in_eng = getattr(nc, cfg["in_eng"])
                xt0 = xpool.tile([kf, pxp + N_LOC], in_dt)
                in_eng.dma_start(xt0[:, : pxp + x_chunk], xTw[:, : pxp + x_chunk])
                for ci in range(1, cfg["x_chunks"]):
                    lo = pxp + ci * x_chunk
                    in_eng.dma_start(xt0[:, lo : lo + x_chunk], xTw[:, lo : lo + x_chunk])
                wt = xt0[:, :pxp]

                def lhs_ap(t):
                    off = pxp + t * TILE
                    return xt0[:, off : off + TILE]

            copy_engines = {
                "s": nc.scalar.copy,
                "v": nc.vector.tensor_copy,
            }
            dma_engines = [getattr(nc, e) for e in cfg["dma_engines"]]

            copy_idx = 0
            for g in range(groups):
                gt = opool.tile([TILE, tiles_per_group * N_PX], f32, tag=f"g{g}")
                for jp in range(tiles_per_group // pack):
                    ps = pspool.tile([TILE, pack * ps_stride], f32)
                    for h in range(pack):
                        t = g * tiles_per_group + jp * pack + h
                        nc.tensor.matmul(
                            ps[:, h * ps_stride : h * ps_stride + pxp],
                            lhs_ap(t),
                            wt,
                            start=True,
                            stop=True,
                        )
                    src = ps[:].rearrange("p (h c) -> p h c", h=pack)[:, :, :N_PX]
                    lo = jp * pack * N_PX
                    dst = gt[:, lo : lo + pack * N_PX].rearrange(
                        "p (h c) -> p h c", h=pack
                    )
                    pat = cfg["copy_pattern"]
                    copy_engines[pat[copy_idx % len(pat)]](dst, src)
                    copy_idx += 1
                if cfg.get("pmajor") and cfg.get("split_dma"):
                    # both HWDGE queues stream halves of the same group
                    # concurrently: halves per-queue blocking time
                    dma_engines[0].dma_start(out_g[g][:64], gt[:64])
                    dma_engines[1].dma_start(out_g[g][64:], gt[64:])
                elif cfg.get("pmajor"):
                    dma_engines[g % len(dma_engines)].dma_start(out_g[g], gt[:])
                else:
                    dma_engines[g % len(dma_engines)].dma_start(
                        out_g[g],
                        gt[:].rearrange("p (j c) -> p j c", j=tiles_per_group),
                    )

    nc.compile()
    return nc


K_F16 = 18  # 16 fp16 features + 2 const-one rows pairing with (-tau) hi/lo planes


def _build_nc_f16(cfg):
    """Samples-on-partitions, single fp16 plane (K=18).

    With rel-err budget 2e-2, a single fp16 plane of the 16-feature
    bilinear form gives Frobenius rel err ~3.4e-4 incl. fp16 output
    rounding (verified vs reference on host). Rows 16/17 are const-one
    against fp16 hi/lo planes of -tau, so tau is exact to ~2^-22.

    Per 128-sample tile: PSUM[128,242] = lhsT(18,128).T @ W(18,242);
    PSUM->SBUF copies downconvert to fp16, alternating vector/scalar;
    each group of `tpg` tiles is DMAed as one contiguous HBM block
    (fp16, ~0.5MB) and the host unscrambles + upcasts. Output HBM
    traffic is 3.87MB/core vs 7.93MB fp32, input 0.3MB vs 1.67MB.
    """
    import concourse.mybir as mybir
    from concourse import bacc
    from concourse.tile import TileContext

    f32 = mybir.dt.float32
    f16 = mybir.dt.float16

    K = K_F16
    ps_stride = 256
    pack = cfg["pack"]
    groups = cfg["groups"]
    tpg = N_TILES // groups
    assert tpg % pack == 0

    nc = bacc.Bacc("TRN2", target_bir_lowering=False, debug=False)
    TileContext = _tc_class(TileContext, cfg)
    xw = nc.declare_dram_parameter("xw", [K, N_PX + N_LOC], f16, isOutput=False)
    out_flat = nc.declare_dram_parameter("out_flat", [N_LOC * N_PX], f16, isOutput=True)

    n_xchunks = cfg["x_chunks"]
    x_chunk = N_LOC // n_xchunks

    with TileContext(nc) as tc:
        with (
            tc.tile_pool(name="xin", bufs=1) as xpool,
            tc.tile_pool(name="ps", bufs=cfg["psum_bufs"], space="PSUM") as pspool,
            tc.tile_pool(name="ob", bufs=1) as opool,
        ):
            in_eng = getattr(nc, cfg["in_eng"])
            xt0 = xpool.tile([K, N_PX + N_LOC], f16)
            # W + first X chunk in one DMA, then the remaining chunks, so
            # early matmuls only wait on the first transfer
            in_eng.dma_start(xt0[:, : N_PX + x_chunk], xw[:, : N_PX + x_chunk])
            for ci in range(1, n_xchunks):
                lo = N_PX + ci * x_chunk
                in_eng.dma_start(xt0[:, lo : lo + x_chunk], xw[:, lo : lo + x_chunk])
            wt = xt0[:, :N_PX]

            def lhs_ap(t):
                off = N_PX + t * TILE
                return xt0[:, off : off + TILE]

            copy_engines = {
                "s": nc.scalar.copy,
                "v": nc.vector.tensor_copy,
            }
            dma_engines = [getattr(nc, e) for e in cfg["dma_engines"]]
            pat = cfg["copy_pattern"]

            copy_idx = 0
            for g in range(groups):
                gt = opool.tile([TILE, tpg * N_PX], f16, tag=f"g{g}")
                for jp in range(tpg // pack):
                    ps = pspool.tile([TILE, pack * ps_stride], f32)
                    for h in range(pack):
                        t = g * tpg + jp * pack + h
                        nc.tensor.matmul(
                            ps[:, h * ps_stride : h * ps_stride + N_PX],
                            lhs_ap(t),
                            wt,
                            start=True,
                            stop=True,
                        )
                    src = ps[:].rearrange("p (h c) -> p h c", h=pack)[:, :, :N_PX]
                    lo = jp * pack * N_PX
                    dst = gt[:, lo : lo + pack * N_PX].rearrange(
                        "p (h c) -> p h c", h=pack
                    )
                    copy_engines[pat[copy_idx % len(pat)]](dst, src)
                    copy_idx += 1
                off = g * TILE * tpg * N_PX
                dest = out_flat[off : off + TILE * tpg * N_PX].rearrange(
                    "(p c) -> p c", p=TILE
                )
                dma_engines[g % len(dma_engines)].dma_start(dest, gt[:])

    nc.compile()
    return nc


def _build_nc_g(cfg):
    """Pixels-on-partitions, single fp16 plane: the PE-efficient layout.

    Layout F pays sem+LDWEIGHTS+MATMUL per 128-sample tile (~400ns x 64 =
    26us serialized on the PE sequencer, cold-clocked). Here the
    STATIONARY operand is the (18, px_w) weight chunk -- reloaded only on
    pixel-chunk switch -- and the moving operand is 512-sample slices:
    32 matmuls of 512 cycles total, back-to-back, so the PE HAM warms.

    Warmup: a few dummy matmuls off a memset tile run during the input
    DMA wait (HAM un-throttle ~3.4us earlier), and a tiny scalar copy
    pulls the one-time ACT_TABLE_LOAD (1.3us) off the critical path.

    PSUM->SBUF copies take pack x 512-sample slices at once (FD=1024
    contiguous across 2 PSUM banks); output staged per group_w samples
    and DMAed as contiguous HBM blocks; host transposes + upcasts.
    """
    import concourse.mybir as mybir
    from concourse import bacc
    from concourse.tile import TileContext

    f32 = mybir.dt.float32
    f16 = mybir.dt.float16

    K = K_F16
    slice_w = cfg["slice_w"]        # samples per matmul (<=512: one PSUM bank)
    pack = cfg["pack"]              # matmuls per PSUM tile / per copy
    group_w = cfg["group_w"]        # samples per output DMA
    spg = group_w // slice_w
    n_groups = N_LOC // group_w
    assert spg % pack == 0

    nc = bacc.Bacc("TRN2", target_bir_lowering=False, debug=False)
    TileContext = _tc_class(TileContext, cfg)
    KG = 16 if cfg.get("tau_bias") else K
    xw = nc.declare_dram_parameter("xw", [KG, N_PX + N_LOC], f16, isOutput=False)
    if cfg.get("tau_bias"):
        taus = nc.declare_dram_parameter("taus", [TILE, 2], f32, isOutput=False)
    out_flat = nc.declare_dram_parameter("out_flat", [N_PX * N_LOC], f16, isOutput=True)

    n_xchunks = cfg["x_chunks"]
    x_chunk = N_LOC // n_xchunks

    with TileContext(nc) as tc:
        with (
            tc.tile_pool(name="xin", bufs=1) as xpool,
            tc.tile_pool(name="ps", bufs=cfg["psum_bufs"], space="PSUM") as pspool,
            tc.tile_pool(name="ob", bufs=1) as opool,
        ):
            in_eng = getattr(nc, cfg["in_eng"])
            xt0 = xpool.tile([KG, N_PX + N_LOC], f16)
            in_eng.dma_start(xt0[:, : N_PX + x_chunk], xw[:, : N_PX + x_chunk])
            for ci in range(1, n_xchunks):
                lo = N_PX + ci * x_chunk
                in_eng.dma_start(xt0[:, lo : lo + x_chunk], xw[:, lo : lo + x_chunk])
            if cfg.get("tau_bias"):
                tt = xpool.tile([TILE, 2], f32, tag="taus")
                in_eng.dma_start(tt[:], taus[:])

            # --- warmup: runs while the input DMA is in flight ---
            warm_mms = cfg.get("warm_mms", 0)
            if warm_mms or cfg.get("warm_tables"):
                with tc.tile_pool(name="psw", bufs=1, space="PSUM") as pswarm:
                    dw = xpool.tile([KG, TILE + slice_w], f16, tag="warm")
                    nc.vector.memset(dw[:], 0.0)
                    dscr = xpool.tile([KG, 32], f16, tag="wscr")
                    nc.scalar.copy(dscr[:], dw[:, :32])  # pulls ACT_TABLE_LOAD early
                    for _ in range(warm_mms):
                        sp = pswarm.tile([TILE, slice_w], f32, tag="warm")
                        nc.tensor.matmul(
                            sp[:], dw[:, :TILE], dw[:, TILE : TILE + slice_w],
                            start=True, stop=True,
                        )

            dma_engines = [getattr(nc, e) for e in cfg["dma_engines"]]
            pat = cfg["copy_pattern"]

            copy_idx = 0
            dma_idx = 0
            px_lo = 0
            for c, px_w in enumerate(PX_CHUNKS):
                wa = xt0[:, px_lo : px_lo + px_w]
                for g in range(n_groups):
                    gt = opool.tile([px_w, group_w], f16, tag=f"g{c}_{g}")
                    for jp in range(spg // pack):
                        ps = pspool.tile([TILE, pack * slice_w], f32)
                        for h in range(pack):
                            s = (g * spg + jp * pack + h) * slice_w
                            xm = xt0[:, N_PX + s : N_PX + s + slice_w]
                            nc.tensor.matmul(
                                ps[:px_w, h * slice_w : (h + 1) * slice_w],
                                wa,
                                xm,
                                start=True,
                                stop=True,
                            )
                        dst = gt[:, jp * pack * slice_w : (jp + 1) * pack * slice_w]
                        eng = pat[copy_idx % len(pat)]
                        if cfg.get("tau_bias"):
                            if eng == "s":
                                nc.scalar.activation(
                                    dst,
                                    ps[:px_w],
                                    mybir.ActivationFunctionType.Identity,
                                    bias=tt[:px_w, c : c + 1],
                                )
                            else:
                                nc.vector.tensor_scalar_add(
                                    dst, ps[:px_w], tt[:px_w, c : c + 1]
                                )
                        elif eng == "s":
                            nc.scalar.copy(dst, ps[:px_w])
                        else:
                            nc.vector.tensor_copy(dst, ps[:px_w])
                        copy_idx += 1
                    off = px_lo * N_LOC + g * px_w * group_w
                    dest = out_flat[off : off + px_w * group_w].rearrange(
                        "(p c) -> p c", p=px_w
                    )
                    dma_engines[dma_idx % len(dma_engines)].dma_start(dest, gt[:])
                    dma_idx += 1
                px_lo += px_w

    nc.compile()
    return nc


def _build_nc_h(cfg):
    """Row-tiled fp16 layout: both pixel chunks stream CONCURRENTLY.

    The PE on this part is clock-capped at 1.2GHz and our K is only 16,
    so the array is row-starved: a full-width matmul uses 16 of 128 rows
    and the sample stream must pass twice (2 pixel chunks) = 16384
    cycles. Row tiling (tile_position) places pixel chunk A (128 px,
    K=16 at array rows 0-15) and chunk B (114 px, rows 32-47) as two
    matmuls that the PE runs CONCURRENTLY in disjoint row groups - one
    pass over samples, 8192 cycles (~7us).

    X is host-replicated at SBUF partitions 0-15 and 32-47; W chunk A
    lives at partitions 0-15, chunk B at 32-47. Each 512-sample slice
    issues two matmuls (tile_position (0,0) / (32,0)) into separate
    PSUM tiles; all 512 PSUM columns are real samples (no 242-padding).
    tau is applied as per-partition bias during the PSUM->SBUF copies.
    """
    import concourse.mybir as mybir
    from concourse import bacc
    from concourse.tile import TileContext

    f32 = mybir.dt.float32
    f16 = mybir.dt.float16

    sw = cfg["slice_w"]
    pack = cfg["pack"]
    group_w = cfg["group_w"]
    spg = group_w // sw
    n_groups = N_LOC // group_w
    assert spg % pack == 0
    PA, PB = PX_CHUNKS  # 128, 114
    WPAD = 128

    nc = bacc.Bacc("TRN2", target_bir_lowering=False, debug=False)
    TileContext = _tc_class(TileContext, cfg)
    xw = nc.declare_dram_parameter("xw", [48, WPAD + N_LOC], f16, isOutput=False)
    taus = nc.declare_dram_parameter("taus", [TILE, 2], f32, isOutput=False)
    out_flat = nc.declare_dram_parameter("out_flat", [N_PX * N_LOC], f16, isOutput=True)

    chunks = cfg["h_chunks"]  # per-row-group X chunk widths, sum = N_LOC

    with TileContext(nc) as tc:
        with (
            tc.tile_pool(name="xin", bufs=1) as xpool,
            tc.tile_pool(name="ps", bufs=cfg["psum_bufs"], space="PSUM") as pspool,
            tc.tile_pool(name="ob", bufs=1) as opool,
        ):
            in_eng = getattr(nc, cfg["in_eng"])
            xt0 = xpool.tile([48, WPAD + N_LOC], f16)
            tt = xpool.tile([TILE, 2], f32, tag="taus")
            in_eng.dma_start(tt[:], taus[:])
            lo = WPAD + 0
            first = True
            for w in chunks:
                for rg in (0, 32):
                    a, b = (0, lo + w) if first else (lo, lo + w)
                    in_eng.dma_start(xt0[rg : rg + 16, a:b], xw[rg : rg + 16, a:b])
                first = False
                lo += w

            if cfg.get("warm_tables"):
                dw = xpool.tile([16, 64], f16, tag="warm")
                nc.vector.memset(dw[:], 0.0)
                dscr = xpool.tile([16, 32], f16, tag="wscr")
                nc.scalar.copy(dscr[:], dw[:, :32])  # pulls ACT_TABLE_LOAD early

            wA = xt0[0:16, 0:PA]
            wB = xt0[32:48, 0:PB]
            dma_engines = [getattr(nc, e) for e in cfg["dma_engines"]]
            pat = cfg["copy_pattern"]

            copy_idx = 0
            dma_idx = 0
            for g in range(n_groups):
                gtA = opool.tile([PA, group_w], f16, tag=f"ga{g}")
                gtB = opool.tile([PB, group_w], f16, tag=f"gb{g}")
                for jp in range(spg // pack):
                    # ONE shared PSUM tile per pack: A and B matmuls land in
                    # alternating banks, so their pool-slot readiness is
                    # identical and the scheduler keeps the A,B,A,B order
                    # that lets the PE overlap them in disjoint row groups.
                    ps = pspool.tile([TILE, 2 * pack * sw], f32)
                    for h in range(pack):
                        s = WPAD + (g * spg + jp * pack + h) * sw
                        nc.tensor.matmul(
                            ps[:PA, (2 * h) * sw : (2 * h + 1) * sw],
                            wA,
                            xt0[0:16, s : s + sw],
                            start=True,
                            stop=True,
                            tile_position=(0, 0),
                        )
                        nc.tensor.matmul(
                            ps[:PB, (2 * h + 1) * sw : (2 * h + 2) * sw],
                            wB,
                            xt0[32:48, s : s + sw],
                            start=True,
                            stop=True,
                            tile_position=(32, 0),
                        )
                    lo = jp * pack * sw
                    ps3 = ps[:].rearrange("p (h two c) -> p h two c", h=pack, two=2)
                    for half, (pxw, gt_t, col) in enumerate(
                        ((PA, gtA, 0), (PB, gtB, 1))
                    ):
                        src = ps3[:pxw, :, half]
                        dst = gt_t[:, lo : lo + pack * sw].rearrange(
                            "p (h c) -> p h c", h=pack
                        )
                        eng = pat[copy_idx % len(pat)]
                        if eng == "s":
                            nc.scalar.activation(
                                dst,
                                src,
                                mybir.ActivationFunctionType.Identity,
                                bias=tt[:pxw, col : col + 1],
                            )
                        else:
                            nc.vector.tensor_scalar_add(
                                dst, src, tt[:pxw, col : col + 1]
                            )
                        copy_idx += 1
                offA = g * PA * group_w
                offB = PA * N_LOC + g * PB * group_w
                for (off, pxw, gt_t) in ((offA, PA, gtA), (offB, PB, gtB)):
                    dest = out_flat[off : off + pxw * group_w].rearrange(
                        "(p c) -> p c", p=pxw
                    )
                    dma_engines[dma_idx % len(dma_engines)].dma_start(dest, gt_t[:])
                    dma_idx += 1

    nc.compile()
    return nc


def _build_nc_h2(cfg):
    """Row-tiled fp16 with single-copy evacuation (the consolidated layout).

    Like H (two concurrent row-group matmuls cover all 242 px in one
    8192-cycle sample pass) but:
      - K=18: tau enters via two const-one rows against (-tau) fp16
        hi/lo planes, so the PSUM->SBUF op is a plain dtype-cast copy;
      - the B chunk's stationary is zero-padded to 128 columns, so both
        matmuls write full 128-partition banks and ONE copy per slice
        evacuates A+B together (junk pixel rows ride along for free -
        partitions are parallel lanes on DVE/ACT);
      - pack=1 with 4 PSUM slots in flight decouples the MM/copy
        pipeline; copies alternate vector/scalar;
      - output ships [128, slices*2*512] fp16 blocks (incl ~8% junk
        rows); host strips them.
    """
    import concourse.mybir as mybir
    from concourse import bacc
    from concourse.tile import TileContext

    f32 = mybir.dt.float32
    f16 = mybir.dt.float16

    K = K_F16  # 18
    sw = cfg["slice_w"]
    group_w = cfg["group_w"]
    spg = group_w // sw
    n_slices = N_LOC // sw
    n_groups = N_LOC // group_w
    WPAD = 128

    nc = bacc.Bacc("TRN2", target_bir_lowering=False, debug=False)
    TileContext = _tc_class(TileContext, cfg)
    xw = nc.declare_dram_parameter("xw", [50, WPAD + N_LOC], f16, isOutput=False)
    out_flat = nc.declare_dram_parameter(
        "out_flat", [TILE * 2 * N_LOC], f16, isOutput=True
    )

    chunks = cfg["h_chunks"]

    with TileContext(nc) as tc:
        with (
            tc.tile_pool(name="xin", bufs=1) as xpool,
            tc.tile_pool(name="ps", bufs=cfg["psum_bufs"], space="PSUM") as pspool,
            tc.tile_pool(name="ob", bufs=1) as opool,
        ):
            in_eng = getattr(nc, cfg["in_eng"])
            in_eng2 = getattr(nc, cfg.get("in_eng2", cfg["in_eng"]))
            xt0 = xpool.tile([32 + K, WPAD + N_LOC], f16)
            if cfg.get("dma_warm"):
                # the first dma_start on an engine pays a ~0.3-0.9us
                # first-issue penalty; absorb it with a tiny transfer so
                # the real chunk0 issues at steady-state cost
                dwm1 = xpool.tile([1, 32], f16, tag="dwm1")
                dwm2 = xpool.tile([1, 32], f16, tag="dwm2")
                in_eng.dma_start(dwm1[:], xw[0:1, 0:32])
                in_eng2.dma_start(dwm2[:], xw[0:1, 0:32])
            lo = WPAD
            first = True
            for w in chunks:
                for rg in (0, 32):
                    a, b = (0, lo + w) if first else (lo, lo + w)
                    # first chunk pair goes out on two engines in parallel
                    # so the opening matmul pair starts ~0.7us earlier
                    eng = in_eng2 if (first and rg == 32) else in_eng
                    eng.dma_start(xt0[rg : rg + K, a:b], xw[rg : rg + K, a:b])
                first = False
                lo += w

            if cfg.get("warm_tables"):
                dw = xpool.tile([16, 64], f16, tag="warm")
                nc.vector.memset(dw[:], 0.0)
                dscr = xpool.tile([16, 32], f16, tag="wscr")
                nc.scalar.copy(dscr[:], dw[:, :32])  # pulls ACT_TABLE_LOAD early

            wA = xt0[0:K, 0:WPAD]
            wB = xt0[32 : 32 + K, 0:WPAD]
            copy_engines = {"s": nc.scalar.copy, "v": nc.vector.tensor_copy}
            dma_engines = [getattr(nc, e) for e in cfg["dma_engines"]]
            pat = cfg["copy_pattern"]

            # group widths in samples; a smaller final group shrinks the
            # fully-exposed last DMA
            gws = list(cfg.get("h_groups") or [group_w] * n_groups)
            assert sum(gws) == N_LOC and all(w % sw == 0 for w in gws)

            s_base = 0
            off = 0
            for g, gw in enumerate(gws):
                spg = gw // sw
                gt = opool.tile([TILE, spg * 2 * sw], f16, tag=f"g{g}")
                for sj in range(spg):
                    sl = s_base // sw + sj
                    s = WPAD + sl * sw
                    ps = pspool.tile([TILE, 2 * sw], f32)
                    nc.tensor.matmul(
                        ps[:, 0:sw],
                        wA,
                        xt0[0:K, s : s + sw],
                        start=True,
                        stop=True,
                        tile_position=(0, 0),
                    )
                    nc.tensor.matmul(
                        ps[:, sw : 2 * sw],
                        wB,
                        xt0[32 : 32 + K, s : s + sw],
                        start=True,
                        stop=True,
                        tile_position=(32, 0),
                    )
                    copy_engines[pat[sl % len(pat)]](
                        gt[:, sj * 2 * sw : (sj + 1) * 2 * sw], ps[:]
                    )
                s_base += gw
                blk_w = spg * 2 * sw
                dest = out_flat[off : off + TILE * blk_w].rearrange(
                    "(p c) -> p c", p=TILE
                )
                dma_engines[g % len(dma_engines)].dma_start(dest, gt[:])
                off += TILE * blk_w

    # LDWEIGHTS dedup: the two stationaries (A at PE rows 0-17, B at rows
    # 32-49) never change and occupy disjoint row groups, so after each is
    # loaded once every further reload the Tile splitter emitted is
    # redundant. Shrink repeats to a single (idempotent) column: LDW cost
    # scales with column count, so they become ~free and the PE pull-ahead
    # hides them entirely.
    if cfg.get("ldw_dedup"):
        seen = set()
        for fn in nc.m.functions:
            for blk in fn.blocks:
                for ins in blk.instructions:
                    if type(ins).__name__ != "InstLdweights":
                        continue
                    ap = ins.ins[0]
                    key = (str(ins.tile_position), ap.concise(), ap.offset)
                    if key in seen:
                        l = ap.ap
                        if len(l) >= 2 and l[-1][1] > 1:
                            ap.ap = list(l[:-1]) + [[l[-1][0], 1]]
                    else:
                        seen.add(key)

    nc.compile()
    return nc


def _pack_host_h2(S_re, S_im, D_re, D_im, tau):
    X16, W16 = _features16(S_re, S_im, D_re, D_im, tau)
    Xh = X16.astype(np.float16)
    Wh = W16.astype(np.float16)
    ntau = -np.asarray(tau, dtype=np.float32)
    th = ntau.astype(np.float16)
    tl = (ntau - th.astype(np.float32)).astype(np.float16)
    PA, PB = PX_CHUNKS
    WPAD = 128

    in_maps = []
    for i in range(N_CORES):
        cols = np.zeros((50, WPAD + N_LOC), dtype=np.float16)
        # group A: px 0-127
        cols[0:16, :PA] = Wh[:, :PA]
        cols[16, :PA] = th[:PA]
        cols[17, :PA] = tl[:PA]
        # group B: px 128-241 (cols 114-127 stay zero)
        cols[32:48, :PB] = Wh[:, PA:]
        cols[48, :PB] = th[PA:]
        cols[49, :PB] = tl[PA:]
        xc = Xh[:, i * N_LOC : (i + 1) * N_LOC]
        cols[0:16, WPAD:] = xc
        cols[16:18, WPAD:] = np.float16(1.0)
        cols[32:48, WPAD:] = xc
        cols[48:50, WPAD:] = np.float16(1.0)
        in_maps.append({"xw": cols})
    return in_maps


def _unpack_host_h2(res, cfg):
    sw = cfg["slice_w"]
    PA, PB = PX_CHUNKS
    gws = list(cfg.get("h_groups") or [cfg["group_w"]] * (N_LOC // cfg["group_w"]))
    out = np.empty((N_SAMPLES, N_PX), dtype=np.float32)
    outT = np.empty((N_PX, N_LOC), dtype=np.float16)
    for i in range(N_CORES):
        buf = np.asarray(res.results[i]["out_flat"])
        # device layout: groups are contiguous, each [128, spg, 2, sw]
        off = 0
        s0 = 0
        for gw in gws:
            spg = gw // sw
            arr = buf[off : off + TILE * spg * 2 * sw].reshape(TILE, spg, 2, sw)
            outT[:PA, s0 : s0 + gw] = arr[:, :, 0, :].reshape(TILE, gw)
            outT[PA:, s0 : s0 + gw] = arr[:PB, :, 1, :].reshape(PB, gw)
            off += TILE * spg * 2 * sw
            s0 += gw
        out[i * N_LOC : (i + 1) * N_LOC] = outT.T.astype(np.float32)
    return out


def _pack_host_h(S_re, S_im, D_re, D_im, tau):
    X16, W16 = _features16(S_re, S_im, D_re, D_im, tau)
    Xh = X16.astype(np.float16)  # (16, N)
    Wh = W16.astype(np.float16)  # (16, 242)
    ntau = -np.asarray(tau, dtype=np.float32)
    PA, PB = PX_CHUNKS
    WPAD = 128

    ts_ = np.zeros((TILE, 2), dtype=np.float32)
    ts_[:PA, 0] = ntau[:PA]
    ts_[:PB, 1] = ntau[PA:]

    in_maps = []
    for i in range(N_CORES):
        cols = np.zeros((48, WPAD + N_LOC), dtype=np.float16)
        cols[0:16, :PA] = Wh[:, :PA]
        cols[32:48, :PB] = Wh[:, PA:]
        xc = Xh[:, i * N_LOC : (i + 1) * N_LOC]
        cols[0:16, WPAD:] = xc
        cols[32:48, WPAD:] = xc
        in_maps.append({"xw": cols, "taus": ts_})
    return in_maps


def _unpack_host_h(res, cfg):
    group_w = cfg["group_w"]
    n_groups = N_LOC // group_w
    PA, PB = PX_CHUNKS
    out = np.empty((N_SAMPLES, N_PX), dtype=np.float32)
    outT = np.empty((N_PX, N_LOC), dtype=np.float16)
    for i in range(N_CORES):
        buf = np.asarray(res.results[i]["out_flat"])
        a = buf[: PA * N_LOC].reshape(n_groups, PA, group_w)
        b = buf[PA * N_LOC :].reshape(n_groups, PB, group_w)
        for g in range(n_groups):
            outT[:PA, g * group_w : (g + 1) * group_w] = a[g]
            outT[PA:, g * group_w : (g + 1) * group_w] = b[g]
        out[i * N_LOC : (i + 1) * N_LOC] = outT.T.astype(np.float32)
    return out


def _unpack_host_g(res, cfg):
    group_w = cfg["group_w"]
    n_groups = N_LOC // group_w
    out = np.empty((N_SAMPLES, N_PX), dtype=np.float32)
    outT = np.empty((N_PX, N_LOC), dtype=np.float16)
    for i in range(N_CORES):
        buf = np.asarray(res.results[i]["out_flat"])
        off = 0
        px_lo = 0
        for px_w in PX_CHUNKS:
            for g in range(n_groups):
                blk = buf[off : off + px_w * group_w].reshape(px_w, group_w)
                outT[px_lo : px_lo + px_w, g * group_w : (g + 1) * group_w] = blk
                off += px_w * group_w
            px_lo += px_w
        out[i * N_LOC : (i + 1) * N_LOC] = outT.T.astype(np.float32)
    return out


def _build_nc_b(cfg):
    """Pixels-on-partitions layout with exact-fp32 bf16 3-way split.

    The bilinear form is reduced to 16 features per sample via the
    Hermitian symmetry of S (4 diag + 6 sym-offdiag + 6 antisym-offdiag,
    off-diagonal weights doubled). X16 = Xh+Xm+Xl, W16 = Wh+Wm+Wl (bf16
    planes); the six significant cross-terms (hh, mh, hm, lh, hl, mm) are
    K-stacked into ONE bf16 matmul with K = 6*16 = 96:
      [Xh;Xm;Xh;Xl;Xh;Xm] x [Wh;Wh;Wm;Wh;Wl;Wm]
    (dropped ml/lm/ll terms are ~2^-27 relative -> fp32-grade accuracy).
    Stationary operand is the W side (reloaded only on pixel-chunk switch),
    moving is samples, so the PE streams at 1 col/cycle. The output lands
    transposed (242 x 8192) and is unscrambled on the host. tau is applied
    per-partition during the PSUM->SBUF copy (ACT Identity bias / DVE
    tensor_scalar add). Pixel chunks are 128 + 114 so output DMAs engage
    all 16 SDMA engines (8 partitions each).
    """
    import concourse.mybir as mybir
    from concourse import bacc
    from concourse.tile import TileContext

    f32 = mybir.dt.float32
    bf16 = mybir.dt.bfloat16

    slice_w = cfg["slice_w"]
    group_w = cfg["group_w"]
    slices_per_group = group_w // slice_w
    n_groups = N_LOC // group_w

    nc = bacc.Bacc("TRN2", target_bir_lowering=False, debug=False)
    TileContext = _tc_class(TileContext, cfg)
    xw = nc.declare_dram_parameter("xw", [96, W_PAD + N_LOC], bf16, isOutput=False)
    taus = nc.declare_dram_parameter("taus", [128, 2], f32, isOutput=False)
    if cfg["linear_out"]:
        # each (px_w, group_w) staging tile lands as one contiguous HBM
        # block -> the DMA splits evenly across all 16 SDMA engines
        out_flat = nc.declare_dram_parameter("out_flat", [N_PX * N_LOC], f32, isOutput=True)
    else:
        outT = nc.declare_dram_parameter("outT", [N_PX, N_LOC], f32, isOutput=True)

    n_xchunks = cfg["x_chunks"]
    x_chunk = N_LOC // n_xchunks

    with TileContext(nc) as tc:
        with (
            tc.tile_pool(name="xin", bufs=1) as xpool,
            tc.tile_pool(name="ps", bufs=cfg["psum_bufs"], space="PSUM") as pspool,
            tc.tile_pool(name="ob", bufs=1) as opool,
        ):
            # W planes + tau first (small, fast), then per-chunk sample
            # tiles so early matmuls only wait on their own chunk's DMA
            in_eng = getattr(nc, cfg["in_eng"])
            wtile = xpool.tile([96, W_PAD], bf16, tag="w")
            in_eng.dma_start(wtile[:], xw[:, :W_PAD])
            tt = xpool.tile([128, 2], f32, tag="taus")
            in_eng.dma_start(tt[:], taus[:])
            xts = []
            for ci in range(n_xchunks):
                xt = xpool.tile([96, x_chunk], bf16, tag=f"x{ci}")
                lo = W_PAD + ci * x_chunk
                in_eng.dma_start(xt[:], xw[:, lo : lo + x_chunk])
                xts.append(xt)

            dma_engines = [getattr(nc, e) for e in cfg["dma_engines"]]
            copy_idx = 0
            dma_idx = 0
            px_lo = 0
            for c, px_w in enumerate(PX_CHUNKS):
                wa = wtile[:, px_lo : px_lo + px_w]
                for g in range(n_groups):
                    gt = opool.tile([px_w, group_w], f32, tag=f"g{c}_{g}")
                    for sj in range(slices_per_group):
                        s = g * slices_per_group + sj
                        ci, off = divmod(s * slice_w, x_chunk)
                        xm = xts[ci][:, off : off + slice_w]
                        ps = pspool.tile([px_w, slice_w], f32, tag="ps")
                        nc.tensor.matmul(ps[:], wa, xm, start=True, stop=True)
                        dst = gt[:, sj * slice_w : (sj + 1) * slice_w]
                        pat = cfg["copy_pattern"]
                        eng = pat[copy_idx % len(pat)]
                        if eng == "s":
                            nc.scalar.activation(
                                dst,
                                ps[:],
                                mybir.ActivationFunctionType.Identity,
                                bias=tt[:px_w, c : c + 1],
                            )
                        else:
                            nc.vector.tensor_scalar_add(
                                dst, ps[:], tt[:px_w, c : c + 1]
                            )
                        copy_idx += 1
                    if cfg["linear_out"]:
                        off = (px_lo * N_LOC) + g * px_w * group_w
                        dest = out_flat[off : off + px_w * group_w].rearrange(
                            "(p c) -> p c", p=px_w
                        )
                    else:
                        dest = outT[
                            px_lo : px_lo + px_w, g * group_w : (g + 1) * group_w
                        ]
                    dma_engines[dma_idx % len(dma_engines)].dma_start(dest, gt[:])
                    dma_idx += 1
                px_lo += px_w

    nc.compile()
    return nc


def _get_nc(cfg=None):
    cfg = dict(DEFAULT_CFG, **(cfg or {}))
    key = tuple(sorted((k, str(v)) for k, v in cfg.items()))
    if key not in _BUILT:
        builder = {
            "B": _build_nc_b,
            "F": _build_nc_f16,
            "G": _build_nc_g,
            "H": _build_nc_h,
            "H2": _build_nc_h2,
        }.get(cfg["layout"], _build_nc)
        _BUILT[key] = (builder(cfg), cfg)
    return _BUILT[key]


def _pack_host(S_re, S_im, D_re, D_im, tau, pxp):
    """Build per-core input maps: weights + transposed feature matrix."""
    Dr = np.asarray(D_re, dtype=np.float32)
    Di = np.asarray(D_im, dtype=np.float32)
    tau = np.asarray(tau, dtype=np.float32)

    Wr = Dr[:, None, :] * Dr[None, :, :] + Di[:, None, :] * Di[None, :, :]
    Wi = Di[:, None, :] * Dr[None, :, :] - Dr[:, None, :] * Di[None, :, :]
    W = np.empty((K_FEAT, pxp), dtype=np.float32)
    W[:, N_PX:] = 0.0
    W[:16, :N_PX] = Wr.reshape(16, N_PX)
    W[16:32, :N_PX] = Wi.reshape(16, N_PX)
    W[32, :N_PX] = -tau

    X = np.empty((K_FEAT, N_SAMPLES), dtype=np.float32)
    X[:16] = np.asarray(S_re, dtype=np.float32).reshape(N_SAMPLES, 16).T
    X[16:32] = np.asarray(S_im, dtype=np.float32).reshape(N_SAMPLES, 16).T
    X[32] = 1.0

    in_maps = []
    for i in range(N_CORES):
        xtw = np.empty((K_FEAT, pxp + N_LOC), dtype=np.float32)
        xtw[:, :pxp] = W
        xtw[:, pxp:] = X[:, i * N_LOC : (i + 1) * N_LOC]
        in_maps.append({"xTw": xtw})
    return in_maps


def _features(S_re, S_im, D_re, D_im, tau):
    """(32, N) feature matrix X32 and (32, 242) weight matrix W32 (fp32)."""
    Dr = np.asarray(D_re, dtype=np.float32)
    Di = np.asarray(D_im, dtype=np.float32)
    Wr = Dr[:, None, :] * Dr[None, :, :] + Di[:, None, :] * Di[None, :, :]
    Wi = Di[:, None, :] * Dr[None, :, :] - Dr[:, None, :] * Di[None, :, :]
    W32 = np.concatenate([Wr.reshape(16, N_PX), Wi.reshape(16, N_PX)], 0)
    X32 = np.empty((32, N_SAMPLES), dtype=np.float32)
    X32[:16] = np.asarray(S_re, dtype=np.float32).reshape(N_SAMPLES, 16).T
    X32[16:] = np.asarray(S_im, dtype=np.float32).reshape(N_SAMPLES, 16).T
    return X32, W32


def _bf16_planes(a):
    import ml_dtypes

    bf = ml_dtypes.bfloat16
    h = a.astype(bf)
    r = a - h.astype(np.float32)
    m = r.astype(bf)
    l = (r - m.astype(np.float32)).astype(bf)
    return h, m, l


def _features16(S_re, S_im, D_re, D_im, tau):
    """(16, N) reduced features and (16, 242) weights using Hermitian
    symmetry: 4 diagonal + 6 sym-offdiag (weight doubled) + 6 antisym-
    offdiag (weight doubled)."""
    Dr = np.asarray(D_re, dtype=np.float32)
    Di = np.asarray(D_im, dtype=np.float32)
    Wr = (Dr[:, None, :] * Dr[None, :, :] + Di[:, None, :] * Di[None, :, :]).reshape(
        16, N_PX
    )
    Wi = (Di[:, None, :] * Dr[None, :, :] - Dr[:, None, :] * Di[None, :, :]).reshape(
        16, N_PX
    )
    W16 = np.concatenate([Wr[IDX_DIAG], 2.0 * Wr[IDX_OFF], 2.0 * Wi[IDX_OFF]], 0)

    Sr = np.asarray(S_re, dtype=np.float32).reshape(N_SAMPLES, 16)
    Si = np.asarray(S_im, dtype=np.float32).reshape(N_SAMPLES, 16)
    X16 = np.empty((16, N_SAMPLES), dtype=np.float32)
    X16[0:4] = Sr[:, IDX_DIAG].T
    X16[4:10] = Sr[:, IDX_OFF].T
    X16[10:16] = Si[:, IDX_OFF].T
    return X16, W16


def _pack_host_f16(S_re, S_im, D_re, D_im, tau, tau_bias=False):
    X16, W16 = _features16(S_re, S_im, D_re, D_im, tau)
    Xh = X16.astype(np.float16)  # (16, N)
    Wh = W16.astype(np.float16)  # (16, 242)
    ntau = -np.asarray(tau, dtype=np.float32)

    if tau_bias:
        # tau applied as a per-partition bias during the PSUM->SBUF op
        ts_ = np.zeros((TILE, 2), dtype=np.float32)
        ts_[: PX_CHUNKS[0], 0] = ntau[: PX_CHUNKS[0]]
        ts_[: PX_CHUNKS[1], 1] = ntau[PX_CHUNKS[0] :]
        in_maps = []
        for i in range(N_CORES):
            cols = np.empty((16, N_PX + N_LOC), dtype=np.float16)
            cols[:, :N_PX] = Wh
            cols[:, N_PX:] = Xh[:, i * N_LOC : (i + 1) * N_LOC]
            in_maps.append({"xw": cols, "taus": ts_})
        return in_maps

    th = ntau.astype(np.float16)
    tl = (ntau - th.astype(np.float32)).astype(np.float16)

    in_maps = []
    for i in range(N_CORES):
        cols = np.empty((K_F16, N_PX + N_LOC), dtype=np.float16)
        cols[:16, :N_PX] = Wh
        cols[16, :N_PX] = th
        cols[17, :N_PX] = tl
        cols[:16, N_PX:] = Xh[:, i * N_LOC : (i + 1) * N_LOC]
        cols[16:, N_PX:] = np.float16(1.0)
        in_maps.append({"xw": cols})
    return in_maps


def _unpack_host_f16(res, groups):
    tpg = N_TILES // groups
    out = np.empty((N_SAMPLES, N_PX), dtype=np.float32)
    for i in range(N_CORES):
        buf = res.results[i]["out_flat"]
        # block layout per group: (partition p, tile-in-group j, col c);
        # tile t = g*tpg + j holds samples t*128 + p
        arr = np.asarray(buf).reshape(groups, TILE, tpg, N_PX)
        out[i * N_LOC : (i + 1) * N_LOC] = (
            arr.transpose(0, 2, 1, 3).reshape(N_LOC, N_PX).astype(np.float32)
        )
    return out


def _pack_host_b(S_re, S_im, D_re, D_im, tau):
    import ml_dtypes

    bf = ml_dtypes.bfloat16
    X16, W16 = _features16(S_re, S_im, D_re, D_im, tau)
    Xh, Xm, Xl = _bf16_planes(X16)
    Wh, Wm, Wl = _bf16_planes(W16)

    # K-stacked pairs: hh, mh, hm, lh, hl, mm
    wA = np.concatenate([Wh, Wh, Wm, Wh, Wl, Wm], 0)  # (96, 242)
    xstack = np.concatenate([Xh, Xm, Xh, Xl, Xh, Xm], 0)  # (96, N)

    taus = np.zeros((128, 2), dtype=np.float32)
    tau = np.asarray(tau, dtype=np.float32)
    taus[: PX_CHUNKS[0], 0] = -tau[: PX_CHUNKS[0]]
    taus[: PX_CHUNKS[1], 1] = -tau[PX_CHUNKS[0] :]

    in_maps = []
    for i in range(N_CORES):
        cols = np.empty((96, W_PAD + N_LOC), dtype=bf)
        cols[:, :W_PAD] = wA
        cols[:, W_PAD:] = xstack[:, i * N_LOC : (i + 1) * N_LOC]
        in_maps.append({"xw": cols, "taus": taus})
    return in_maps


def _pack_host_a16(S_re, S_im, D_re, D_im, tau, pmajor=False):
    import ml_dtypes

    bf = ml_dtypes.bfloat16
    X16, W16 = _features16(S_re, S_im, D_re, D_im, tau)
    Xh, Xm, Xl = _bf16_planes(X16)
    Wh, Wm, Wl = _bf16_planes(W16)
    th, tm, tl = _bf16_planes(-np.asarray(tau, dtype=np.float32)[None, :])

    ones = np.ones((1, N_SAMPLES), dtype=np.float32).astype(bf)
    xstack = np.concatenate([Xh, Xm, Xh, Xl, Xh, Xm, ones, ones, ones], 0)  # (99, N)
    wstack = np.concatenate([Wh, Wh, Wm, Wh, Wl, Wm, th, tm, tl], 0)        # (99, 242)

    in_maps = []
    for i in range(N_CORES):
        xcore = xstack[:, i * N_LOC : (i + 1) * N_LOC]
        if pmajor:
            # device tile t partition p holds sample p*64 + t
            xcore = np.ascontiguousarray(
                xcore.reshape(99, TILE, N_TILES).swapaxes(1, 2).reshape(99, N_LOC)
            )
        cols = np.empty((99, N_PX + N_LOC), dtype=bf)
        cols[:, :N_PX] = wstack
        cols[:, N_PX:] = xcore
        in_maps.append({"xTw": cols})
    return in_maps


def _run(inputs, trace=False, cfg=None):
    from concourse.bass_utils import run_bass_kernel_spmd

    nc, full_cfg = _get_nc(cfg)
    if full_cfg["layout"] == "H2":
        in_maps = _pack_host_h2(**inputs)
        res = run_bass_kernel_spmd(nc, in_maps, list(range(N_CORES)), trace=trace)
        out = _unpack_host_h2(res, full_cfg)
        return out, res
    if full_cfg["layout"] == "H":
        in_maps = _pack_host_h(**inputs)
        res = run_bass_kernel_spmd(nc, in_maps, list(range(N_CORES)), trace=trace)
        out = _unpack_host_h(res, full_cfg)
        return out, res
    if full_cfg["layout"] == "G":
        in_maps = _pack_host_f16(**inputs, tau_bias=full_cfg.get("tau_bias", False))
        res = run_bass_kernel_spmd(nc, in_maps, list(range(N_CORES)), trace=trace)
        out = _unpack_host_g(res, full_cfg)
        return out, res
    if full_cfg["layout"] == "F":
        in_maps = _pack_host_f16(**inputs)
        res = run_bass_kernel_spmd(nc, in_maps, list(range(N_CORES)), trace=trace)
        out = _unpack_host_f16(res, full_cfg["groups"])
        return out, res
    if full_cfg["layout"] == "B":
        in_maps = _pack_host_b(**inputs)
        res = run_bass_kernel_spmd(nc, in_maps, list(range(N_CORES)), trace=trace)
        out = np.empty((N_SAMPLES, N_PX), dtype=np.float32)
        n_groups = N_LOC // full_cfg["group_w"]
        for i in range(N_CORES):
            if full_cfg["linear_out"]:
                buf = res.results[i]["out_flat"]
                outT = np.empty((N_PX, N_LOC), dtype=np.float32)
                off = 0
                px_lo = 0
                for px_w in PX_CHUNKS:
                    gw = full_cfg["group_w"]
                    for g in range(n_groups):
                        blk = buf[off : off + px_w * gw].reshape(px_w, gw)
                        outT[px_lo : px_lo + px_w, g * gw : (g + 1) * gw] = blk
                        off += px_w * gw
                    px_lo += px_w
            else:
                outT = res.results[i]["outT"]
            out[i * N_LOC : (i + 1) * N_LOC] = outT.T
    elif full_cfg.get("bf16split"):
        in_maps = _pack_host_a16(**inputs, pmajor=full_cfg.get("pmajor", False))
        res = run_bass_kernel_spmd(nc, in_maps, list(range(N_CORES)), trace=trace)
        out = np.concatenate(
            [res.results[i]["out"] for i in range(N_CORES)], axis=0
        )
        return out, res
    else:
        pxp = 256 if full_cfg["f32r"] else N_PX
        in_maps = _pack_host(**inputs, pxp=pxp)
        res = run_bass_kernel_spmd(nc, in_maps, list(range(N_CORES)), trace=trace)
        out = np.concatenate(
            [res.results[i]["out"] for i in range(N_CORES)], axis=0
        )
    return out, res


def kernel(**inputs) -> np.ndarray:
    out, _ = _run(inputs, trace=False)
    return out



# revision 54
# speedup vs baseline: 1.0711x; 1.0711x over previous
"""Trainium2 Bass kernel for nn_BackProjLayer.

Math: the reference computes, per sample n,
    eigh(S) -> (lam, V);  G = V @ diag(sqrt(max(lam,0)));  y = D^H G
    out[n,p] = sum_d |y[p,d]|^2 - tau[p] = [D^H S_plus D]_pp - tau[p]
Since S = A A^H / Nch is Hermitian PSD by construction, S_plus == S up to
float32 eigensolver noise, so no eigendecomposition is needed:
    out[n,p] = Re(d_p^H S[n] d_p) - tau[p]
With S = Sr + i Si (Sr sym, Si antisym) and d = dr + i di this is a real
bilinear form; by Hermitian symmetry it reduces to 16 features per sample
(4 diag Sr, 6 offdiag Sr doubled, 6 offdiag Si doubled):
    out = X16.T @ W16 - tau     X16 (16, N), W16 (16, 242)

Device kernel (default layout "H2", single fp16 plane):
  - fp16 inputs/outputs: with the 2e-2 rel-err budget, one fp16 plane of
    X16/W16 plus fp16 output rounding gives 3.4e-4 Frobenius rel err;
    tau enters exactly via two const-one rows against (-tau) hi/lo fp16
    planes (K=18). Output ships as fp16 and the host upcasts, halving
    HBM write traffic vs fp32.
  - pixels on PSUM partitions, samples on the moving dim. The PE here is
    clock-capped at 1.2 GHz and K=18 uses 18 of 128 array rows, so the
    kernel row-tiles: pixel chunk A (px 0-127, array rows 0-17,
    tile_position (0,0)) and chunk B (px 128-241 zero-padded to 128
    stationary cols, rows 32-49, tile_position (32,0)) run as two
    CONCURRENT matmuls per 512-sample slice - one pass over samples
    instead of two. X is host-replicated at SBUF partitions 0-17/32-49.
  - both matmuls of a slice write one shared 2-bank PSUM tile; a single
    plain fp16-cast copy (alternating vector/scalar) evacuates A+B
    together (B's junk pixel rows ride free - partitions are parallel
    lanes); 8 grouped ~0.5MB contiguous output DMAs alternate the two
    HWDGE queues (sync/scalar); host strips junk rows and transposes.
  - lightweight Tile exit (tail="no_reset"): the explicit sync.drain()
    already guarantees DMA completion, so the gpsimd dma_reset is
    skipped and only the semaphore clears run.

Sharding: pure data parallel over N across 8 cores (8192 samples/core);
host packs per-core inputs, device returns fp16 blocks per core, host
unscrambles, concatenates and upcasts to fp32.
"""

import sys

for _p in ("/opt/trn_rl_repo", "/root/.axon_site/_ro/trn_rl_repo"):
    if _p not in sys.path:
        sys.path.insert(0, _p)

import numpy as np

N_SAMPLES = 65536
N_CH = 4
N_PX = 242
N_CORES = 8
N_LOC = N_SAMPLES // N_CORES  # 8192

K_FEAT = 2 * N_CH * N_CH + 1  # 33

TILE = 128
N_TILES = N_LOC // TILE  # 64

DEFAULT_CFG = dict(
    layout="H2",     # H2: fp16 row-tiled, single-copy evac; H: fp16 row-tiled;
                     # G: fp16 pixels-on-partitions; F: fp16 samples-on-partitions;
                     # A: bf16split samples-on-partitions; B: bf16split pixels-on-partitions
    bf16split=True,  # exact-fp32 via K-stacked bf16 3-way split (K=99)
    f32r=True,       # (non-split A only) float32r matmul
    pack=2,          # matmul outputs per PSUM tile / per copy instruction
    psum_bufs=4,     # PSUM pool slots (H2: 4 x 2-bank slots = full PSUM)
    groups=8,        # F/A: number of output DMAs
    dma_engines=("sync", "scalar"),  # round-robin for output DMAs
    copy_pattern="vs",  # per-copy engine cycle: s=scalar, v=vector
    x_chunks=4,      # input DMA chunks
    h_chunks=(512, 3584, 4096),  # H/H2: per-row-group X chunk widths
    slice_w=512,     # B/G: samples per matmul (moving dim)
    group_w=1024,    # B/G/H2: samples per output DMA group
    warm_mms=0,      # G: dummy matmuls during the input-DMA wait (PE is
                     # clock-capped at 1.2GHz on this part, so HAM warmup
                     # only delays real work)
    warm_tables=True,  # G: tiny scalar op to pull ACT_TABLE_LOAD early
    tau_bias=True,   # G: K=16, tau as per-partition bias in the copies
    tail="no_reset", # H2 default; see _tc_class
    ldw_dedup=False, # H2: shrinking redundant LDWEIGHTS to 1 column is
                     # numerically safe but measured ~+1.5us at the system
                     # level (nominal-clock runs: 31.1-31.3us vs 29.5-29.8
                     # without) — the rewrite perturbs the schedule timing
    h_groups=None,   # H2: output DMA group widths; None = uniform group_w.
                     # Uneven tails measured a wash: the extra issue slots
                     # land between the final copies and eat the drain gain.
    dma_warm=False,  # H2: dummy first DMA did NOT absorb the issue penalty,
                     # it just delayed chunk0 (~-0.7us)
    split_last_copy=True,  # H2: final copy's halves run on both engines
    in_eng="sync",   # engine issuing input DMAs
    in_eng2="sync",  # H2: engine for the second first-chunk DMA (scalar's
                     # first D2D costs ~1.6us, so parallel issue gains nothing)
    linear_out=True, # B: write output groups as contiguous HBM blocks
    pmajor=True,     # A: partition-major sample mapping (contiguous 7.7KB
                     # HBM runs per partition in output DMAs)
)

# B layout constants
PX_CHUNKS = (128, 114)  # pixels per chunk (DMA uses 8 partitions/SDMA engine,
                        # so a 128-partition transfer engages all 16 engines)
W_PAD = 242          # leading cols of xw holding the stacked W planes
IDX_DIAG = [0, 5, 10, 15]       # S[c,c] positions in the c*4+c' flattening
IDX_OFF = [1, 2, 3, 6, 7, 11]   # S[c,c'] c<c' positions

_BUILT = {}


def _tc_class(base, cfg):
    """Optionally lighten the Tile kernel-tail: keep the drain (output DMA
    completion) and the semaphore clears (needed for NEFF re-execution),
    but trim barrier work per cfg['tail'] mode."""
    mode = cfg.get("tail", "full")
    if mode == "full":
        return base

    from concourse.vector_clock import ScopedClock

    class _TC(base):
        def _drain_and_barrier(self, tick_clock, wait_clock):
            nc = self.nc
            drain_inst = nc.sync.drain()
            wait_clock.add_sem_waits(
                drain_inst.ins, ScopedClock({None: tick_clock.global_clock})
            )
            if mode in ("sem_only", "no_reset"):
                nc.all_engine_barrier(sem_only=True)
            else:
                nc.all_engine_barrier()
            popped = nc._tile_sem_poison_stack.pop()
            assert popped is self._sem_poison
            sems = list(self.sems.allocated().values())
            if mode == "no_reset":
                # the sync.drain() above already guarantees every DMA
                # completed, so the gpsimd dma_reset drain is redundant;
                # just zero the sems for NEFF re-execution
                from concourse.bass import compact_to_ranges

                sem_nums = [s.num if hasattr(s, "num") else s for s in sems]
                for sem_range in compact_to_ranges(sem_nums):
                    nc.gpsimd.sem_clear(sem_range)
                nc._state.prepend_free_semaphores(sem_nums)
                for poison_set in nc._tile_sem_poison_stack:
                    poison_set.update(sem_nums)
            else:
                nc.clear_and_free_semaphores(sems)
            if mode not in ("no2nd", "sem_only", "no_reset"):
                nc.all_engine_barrier()

    return _TC


def _build_nc(cfg):
    import concourse.mybir as mybir
    from concourse import bacc
    from concourse.tile import TileContext

    f32 = mybir.dt.float32
    f32r = mybir.dt.float32r
    bf16 = mybir.dt.bfloat16

    if cfg.get("bf16split"):
        # exact-fp32 bf16 3-way split (see _build_nc_b docstring), with two
        # extra const-one rows pairing against the -tau bf16 planes: K=99
        pxp = N_PX
        in_dt = bf16
        kf = 99
    else:
        pxp = 256 if cfg["f32r"] else N_PX
        in_dt = f32r if cfg["f32r"] else f32
        kf = K_FEAT
    ps_stride = 256
    pack = cfg["pack"]
    groups = cfg["groups"]
    tiles_per_group = N_TILES // groups
    assert tiles_per_group % pack == 0 or pack % tiles_per_group == 0

    # Bacc (not plain Bass): its compile() lowers multi-wait sync_infos into
    # chained EventSemaphores (TRN2 allows 1 wait/instruction).
    nc = bacc.Bacc("TRN2", target_bir_lowering=False, debug=False)
    TileContext = _tc_class(TileContext, cfg)
    xTw = nc.declare_dram_parameter("xTw", [kf, pxp + N_LOC], in_dt, isOutput=False)
    out = nc.declare_dram_parameter("out", [N_LOC, N_PX], f32, isOutput=True)

    if cfg.get("pmajor"):
        # partition-major sample mapping: tile t, partition p <-> sample
        # n = p*64 + t. Each partition's 8-tile group lands in 8
        # CONSECUTIVE output rows -> 7744B contiguous HBM runs per
        # partition (vs 968B strided), much better SDMA descriptor
        # efficiency. Host permutes the input columns to match.
        out_g = out.rearrange(
            "(p g j) c -> g p (j c)", p=TILE, g=groups, j=tiles_per_group
        )
    else:
        out_g = out.rearrange("(g j p) c -> g p j c", p=TILE, j=tiles_per_group)
    x_chunk = N_LOC // cfg["x_chunks"]

    with TileContext(nc) as tc:
        with (
            tc.tile_pool(name="xin", bufs=1) as xpool,
            tc.tile_pool(name="ps", bufs=cfg["psum_bufs"], space="PSUM") as pspool,
            tc.tile_pool(name="ob", bufs=1) as opool,
        ):
            if cfg.get("sep_in"):
                in_eng = getattr(nc, cfg["in_eng"])
                wt_tile = xpool.tile([kf, pxp], in_dt, tag="w")
                in_eng.dma_start(wt_tile[:], xTw[:, :pxp])
                wt = wt_tile[:]
                xts = []
                for ci in range(cfg["x_chunks"]):
                    xt = xpool.tile([kf, x_chunk], in_dt, tag=f"x{ci}")
                    lo = pxp + ci * x_chunk
                    in_eng.dma_start(xt[:], xTw[:, lo : lo + x_chunk])
                    xts.append(xt)

                def lhs_ap(t):
                    ci, off = divmod(t * TILE, x_chunk)
                    return xts[ci][:, off : off + TILE]
            else:
                in_eng = getattr(nc, cfg["in_eng"])
                xt0 = xpool.tile([kf, pxp + N_LOC], in_dt)
                in_eng.dma_start(xt0[:, : pxp + x_chunk], xTw[:, : pxp + x_chunk])
                for ci in range(1, cfg["x_chunks"]):
                    lo = pxp + ci * x_chunk
                    in_eng.dma_start(xt0[:, lo : lo + x_chunk], xTw[:, lo : lo + x_chunk])
                wt = xt0[:, :pxp]

                def lhs_ap(t):
                    off = pxp + t * TILE
                    return xt0[:, off : off + TILE]

            copy_engines = {
                "s": nc.scalar.copy,
                "v": nc.vector.tensor_copy,
            }
            dma_engines = [getattr(nc, e) for e in cfg["dma_engines"]]

            copy_idx = 0
            for g in range(groups):
                gt = opool.tile([TILE, tiles_per_group * N_PX], f32, tag=f"g{g}")
                for jp in range(tiles_per_group // pack):
                    ps = pspool.tile([TILE, pack * ps_stride], f32)
                    for h in range(pack):
                        t = g * tiles_per_group + jp * pack + h
                        nc.tensor.matmul(
                            ps[:, h * ps_stride : h * ps_stride + pxp],
                            lhs_ap(t),
                            wt,
                            start=True,
                            stop=True,
                        )
                    src = ps[:].rearrange("p (h c) -> p h c", h=pack)[:, :, :N_PX]
                    lo = jp * pack * N_PX
                    dst = gt[:, lo : lo + pack * N_PX].rearrange(
                        "p (h c) -> p h c", h=pack
                    )
                    pat = cfg["copy_pattern"]
                    copy_engines[pat[copy_idx % len(pat)]](dst, src)
                    copy_idx += 1
                if cfg.get("pmajor") and cfg.get("split_dma"):
                    # both HWDGE queues stream halves of the same group
                    # concurrently: halves per-queue blocking time
                    dma_engines[0].dma_start(out_g[g][:64], gt[:64])
                    dma_engines[1].dma_start(out_g[g][64:], gt[64:])
                elif cfg.get("pmajor"):
                    dma_engines[g % len(dma_engines)].dma_start(out_g[g], gt[:])
                else:
                    dma_engines[g % len(dma_engines)].dma_start(
                        out_g[g],
                        gt[:].rearrange("p (j c) -> p j c", j=tiles_per_group),
                    )

    nc.compile()
    return nc


K_F16 = 18  # 16 fp16 features + 2 const-one rows pairing with (-tau) hi/lo planes


def _build_nc_f16(cfg):
    """Samples-on-partitions, single fp16 plane (K=18).

    With rel-err budget 2e-2, a single fp16 plane of the 16-feature
    bilinear form gives Frobenius rel err ~3.4e-4 incl. fp16 output
    rounding (verified vs reference on host). Rows 16/17 are const-one
    against fp16 hi/lo planes of -tau, so tau is exact to ~2^-22.

    Per 128-sample tile: PSUM[128,242] = lhsT(18,128).T @ W(18,242);
    PSUM->SBUF copies downconvert to fp16, alternating vector/scalar;
    each group of `tpg` tiles is DMAed as one contiguous HBM block
    (fp16, ~0.5MB) and the host unscrambles + upcasts. Output HBM
    traffic is 3.87MB/core vs 7.93MB fp32, input 0.3MB vs 1.67MB.
    """
    import concourse.mybir as mybir
    from concourse import bacc
    from concourse.tile import TileContext

    f32 = mybir.dt.float32
    f16 = mybir.dt.float16

    K = K_F16
    ps_stride = 256
    pack = cfg["pack"]
    groups = cfg["groups"]
    tpg = N_TILES // groups
    assert tpg % pack == 0

    nc = bacc.Bacc("TRN2", target_bir_lowering=False, debug=False)
    TileContext = _tc_class(TileContext, cfg)
    xw = nc.declare_dram_parameter("xw", [K, N_PX + N_LOC], f16, isOutput=False)
    out_flat = nc.declare_dram_parameter("out_flat", [N_LOC * N_PX], f16, isOutput=True)

    n_xchunks = cfg["x_chunks"]
    x_chunk = N_LOC // n_xchunks

    with TileContext(nc) as tc:
        with (
            tc.tile_pool(name="xin", bufs=1) as xpool,
            tc.tile_pool(name="ps", bufs=cfg["psum_bufs"], space="PSUM") as pspool,
            tc.tile_pool(name="ob", bufs=1) as opool,
        ):
            in_eng = getattr(nc, cfg["in_eng"])
            xt0 = xpool.tile([K, N_PX + N_LOC], f16)
            # W + first X chunk in one DMA, then the remaining chunks, so
            # early matmuls only wait on the first transfer
            in_eng.dma_start(xt0[:, : N_PX + x_chunk], xw[:, : N_PX + x_chunk])
            for ci in range(1, n_xchunks):
                lo = N_PX + ci * x_chunk
                in_eng.dma_start(xt0[:, lo : lo + x_chunk], xw[:, lo : lo + x_chunk])
            wt = xt0[:, :N_PX]

            def lhs_ap(t):
                off = N_PX + t * TILE
                return xt0[:, off : off + TILE]

            copy_engines = {
                "s": nc.scalar.copy,
                "v": nc.vector.tensor_copy,
            }
            dma_engines = [getattr(nc, e) for e in cfg["dma_engines"]]
            pat = cfg["copy_pattern"]

            copy_idx = 0
            for g in range(groups):
                gt = opool.tile([TILE, tpg * N_PX], f16, tag=f"g{g}")
                for jp in range(tpg // pack):
                    ps = pspool.tile([TILE, pack * ps_stride], f32)
                    for h in range(pack):
                        t = g * tpg + jp * pack + h
                        nc.tensor.matmul(
                            ps[:, h * ps_stride : h * ps_stride + N_PX],
                            lhs_ap(t),
                            wt,
                            start=True,
                            stop=True,
                        )
                    src = ps[:].rearrange("p (h c) -> p h c", h=pack)[:, :, :N_PX]
                    lo = jp * pack * N_PX
                    dst = gt[:, lo : lo + pack * N_PX].rearrange(
                        "p (h c) -> p h c", h=pack
                    )
                    copy_engines[pat[copy_idx % len(pat)]](dst, src)
                    copy_idx += 1
                off = g * TILE * tpg * N_PX
                dest = out_flat[off : off + TILE * tpg * N_PX].rearrange(
                    "(p c) -> p c", p=TILE
                )
                dma_engines[g % len(dma_engines)].dma_start(dest, gt[:])

    nc.compile()
    return nc


def _build_nc_g(cfg):
    """Pixels-on-partitions, single fp16 plane: the PE-efficient layout.

    Layout F pays sem+LDWEIGHTS+MATMUL per 128-sample tile (~400ns x 64 =
    26us serialized on the PE sequencer, cold-clocked). Here the
    STATIONARY operand is the (18, px_w) weight chunk -- reloaded only on
    pixel-chunk switch -- and the moving operand is 512-sample slices:
    32 matmuls of 512 cycles total, back-to-back, so the PE HAM warms.

    Warmup: a few dummy matmuls off a memset tile run during the input
    DMA wait (HAM un-throttle ~3.4us earlier), and a tiny scalar copy
    pulls the one-time ACT_TABLE_LOAD (1.3us) off the critical path.

    PSUM->SBUF copies take pack x 512-sample slices at once (FD=1024
    contiguous across 2 PSUM banks); output staged per group_w samples
    and DMAed as contiguous HBM blocks; host transposes + upcasts.
    """
    import concourse.mybir as mybir
    from concourse import bacc
    from concourse.tile import TileContext

    f32 = mybir.dt.float32
    f16 = mybir.dt.float16

    K = K_F16
    slice_w = cfg["slice_w"]        # samples per matmul (<=512: one PSUM bank)
    pack = cfg["pack"]              # matmuls per PSUM tile / per copy
    group_w = cfg["group_w"]        # samples per output DMA
    spg = group_w // slice_w
    n_groups = N_LOC // group_w
    assert spg % pack == 0

    nc = bacc.Bacc("TRN2", target_bir_lowering=False, debug=False)
    TileContext = _tc_class(TileContext, cfg)
    KG = 16 if cfg.get("tau_bias") else K
    xw = nc.declare_dram_parameter("xw", [KG, N_PX + N_LOC], f16, isOutput=False)
    if cfg.get("tau_bias"):
        taus = nc.declare_dram_parameter("taus", [TILE, 2], f32, isOutput=False)
    out_flat = nc.declare_dram_parameter("out_flat", [N_PX * N_LOC], f16, isOutput=True)

    n_xchunks = cfg["x_chunks"]
    x_chunk = N_LOC // n_xchunks

    with TileContext(nc) as tc:
        with (
            tc.tile_pool(name="xin", bufs=1) as xpool,
            tc.tile_pool(name="ps", bufs=cfg["psum_bufs"], space="PSUM") as pspool,
            tc.tile_pool(name="ob", bufs=1) as opool,
        ):
            in_eng = getattr(nc, cfg["in_eng"])
            xt0 = xpool.tile([KG, N_PX + N_LOC], f16)
            in_eng.dma_start(xt0[:, : N_PX + x_chunk], xw[:, : N_PX + x_chunk])
            for ci in range(1, n_xchunks):
                lo = N_PX + ci * x_chunk
                in_eng.dma_start(xt0[:, lo : lo + x_chunk], xw[:, lo : lo + x_chunk])
            if cfg.get("tau_bias"):
                tt = xpool.tile([TILE, 2], f32, tag="taus")
                in_eng.dma_start(tt[:], taus[:])

            # --- warmup: runs while the input DMA is in flight ---
            warm_mms = cfg.get("warm_mms", 0)
            if warm_mms or cfg.get("warm_tables"):
                with tc.tile_pool(name="psw", bufs=1, space="PSUM") as pswarm:
                    dw = xpool.tile([KG, TILE + slice_w], f16, tag="warm")
                    nc.vector.memset(dw[:], 0.0)
                    dscr = xpool.tile([KG, 32], f16, tag="wscr")
                    nc.scalar.copy(dscr[:], dw[:, :32])  # pulls ACT_TABLE_LOAD early
                    for _ in range(warm_mms):
                        sp = pswarm.tile([TILE, slice_w], f32, tag="warm")
                        nc.tensor.matmul(
                            sp[:], dw[:, :TILE], dw[:, TILE : TILE + slice_w],
                            start=True, stop=True,
                        )

            dma_engines = [getattr(nc, e) for e in cfg["dma_engines"]]
            pat = cfg["copy_pattern"]

            copy_idx = 0
            dma_idx = 0
            px_lo = 0
            for c, px_w in enumerate(PX_CHUNKS):
                wa = xt0[:, px_lo : px_lo + px_w]
                for g in range(n_groups):
                    gt = opool.tile([px_w, group_w], f16, tag=f"g{c}_{g}")
                    for jp in range(spg // pack):
                        ps = pspool.tile([TILE, pack * slice_w], f32)
                        for h in range(pack):
                            s = (g * spg + jp * pack + h) * slice_w
                            xm = xt0[:, N_PX + s : N_PX + s + slice_w]
                            nc.tensor.matmul(
                                ps[:px_w, h * slice_w : (h + 1) * slice_w],
                                wa,
                                xm,
                                start=True,
                                stop=True,
                            )
                        dst = gt[:, jp * pack * slice_w : (jp + 1) * pack * slice_w]
                        eng = pat[copy_idx % len(pat)]
                        if cfg.get("tau_bias"):
                            if eng == "s":
                                nc.scalar.activation(
                                    dst,
                                    ps[:px_w],
                                    mybir.ActivationFunctionType.Identity,
                                    bias=tt[:px_w, c : c + 1],
                                )
                            else:
                                nc.vector.tensor_scalar_add(
                                    dst, ps[:px_w], tt[:px_w, c : c + 1]
                                )
                        elif eng == "s":
                            nc.scalar.copy(dst, ps[:px_w])
                        else:
                            nc.vector.tensor_copy(dst, ps[:px_w])
                        copy_idx += 1
                    off = px_lo * N_LOC + g * px_w * group_w
                    dest = out_flat[off : off + px_w * group_w].rearrange(
                        "(p c) -> p c", p=px_w
                    )
                    dma_engines[dma_idx % len(dma_engines)].dma_start(dest, gt[:])
                    dma_idx += 1
                px_lo += px_w

    nc.compile()
    return nc


def _build_nc_h(cfg):
    """Row-tiled fp16 layout: both pixel chunks stream CONCURRENTLY.

    The PE on this part is clock-capped at 1.2GHz and our K is only 16,
    so the array is row-starved: a full-width matmul uses 16 of 128 rows
    and the sample stream must pass twice (2 pixel chunks) = 16384
    cycles. Row tiling (tile_position) places pixel chunk A (128 px,
    K=16 at array rows 0-15) and chunk B (114 px, rows 32-47) as two
    matmuls that the PE runs CONCURRENTLY in disjoint row groups - one
    pass over samples, 8192 cycles (~7us).

    X is host-replicated at SBUF partitions 0-15 and 32-47; W chunk A
    lives at partitions 0-15, chunk B at 32-47. Each 512-sample slice
    issues two matmuls (tile_position (0,0) / (32,0)) into separate
    PSUM tiles; all 512 PSUM columns are real samples (no 242-padding).
    tau is applied as per-partition bias during the PSUM->SBUF copies.
    """
    import concourse.mybir as mybir
    from concourse import bacc
    from concourse.tile import TileContext

    f32 = mybir.dt.float32
    f16 = mybir.dt.float16

    sw = cfg["slice_w"]
    pack = cfg["pack"]
    group_w = cfg["group_w"]
    spg = group_w // sw
    n_groups = N_LOC // group_w
    assert spg % pack == 0
    PA, PB = PX_CHUNKS  # 128, 114
    WPAD = 128

    nc = bacc.Bacc("TRN2", target_bir_lowering=False, debug=False)
    TileContext = _tc_class(TileContext, cfg)
    xw = nc.declare_dram_parameter("xw", [48, WPAD + N_LOC], f16, isOutput=False)
    taus = nc.declare_dram_parameter("taus", [TILE, 2], f32, isOutput=False)
    out_flat = nc.declare_dram_parameter("out_flat", [N_PX * N_LOC], f16, isOutput=True)

    chunks = cfg["h_chunks"]  # per-row-group X chunk widths, sum = N_LOC

    with TileContext(nc) as tc:
        with (
            tc.tile_pool(name="xin", bufs=1) as xpool,
            tc.tile_pool(name="ps", bufs=cfg["psum_bufs"], space="PSUM") as pspool,
            tc.tile_pool(name="ob", bufs=1) as opool,
        ):
            in_eng = getattr(nc, cfg["in_eng"])
            xt0 = xpool.tile([48, WPAD + N_LOC], f16)
            tt = xpool.tile([TILE, 2], f32, tag="taus")
            in_eng.dma_start(tt[:], taus[:])
            lo = WPAD + 0
            first = True
            for w in chunks:
                for rg in (0, 32):
                    a, b = (0, lo + w) if first else (lo, lo + w)
                    in_eng.dma_start(xt0[rg : rg + 16, a:b], xw[rg : rg + 16, a:b])
                first = False
                lo += w

            if cfg.get("warm_tables"):
                dw = xpool.tile([16, 64], f16, tag="warm")
                nc.vector.memset(dw[:], 0.0)
                dscr = xpool.tile([16, 32], f16, tag="wscr")
                nc.scalar.copy(dscr[:], dw[:, :32])  # pulls ACT_TABLE_LOAD early

            wA = xt0[0:16, 0:PA]
            wB = xt0[32:48, 0:PB]
            dma_engines = [getattr(nc, e) for e in cfg["dma_engines"]]
            pat = cfg["copy_pattern"]

            copy_idx = 0
            dma_idx = 0
            for g in range(n_groups):
                gtA = opool.tile([PA, group_w], f16, tag=f"ga{g}")
                gtB = opool.tile([PB, group_w], f16, tag=f"gb{g}")
                for jp in range(spg // pack):
                    # ONE shared PSUM tile per pack: A and B matmuls land in
                    # alternating banks, so their pool-slot readiness is
                    # identical and the scheduler keeps the A,B,A,B order
                    # that lets the PE overlap them in disjoint row groups.
                    ps = pspool.tile([TILE, 2 * pack * sw], f32)
                    for h in range(pack):
                        s = WPAD + (g * spg + jp * pack + h) * sw
                        nc.tensor.matmul(
                            ps[:PA, (2 * h) * sw : (2 * h + 1) * sw],
                            wA,
                            xt0[0:16, s : s + sw],
                            start=True,
                            stop=True,
                            tile_position=(0, 0),
                        )
                        nc.tensor.matmul(
                            ps[:PB, (2 * h + 1) * sw : (2 * h + 2) * sw],
                            wB,
                            xt0[32:48, s : s + sw],
                            start=True,
                            stop=True,
                            tile_position=(32, 0),
                        )
                    lo = jp * pack * sw
                    ps3 = ps[:].rearrange("p (h two c) -> p h two c", h=pack, two=2)
                    for half, (pxw, gt_t, col) in enumerate(
                        ((PA, gtA, 0), (PB, gtB, 1))
                    ):
                        src = ps3[:pxw, :, half]
                        dst = gt_t[:, lo : lo + pack * sw].rearrange(
                            "p (h c) -> p h c", h=pack
                        )
                        eng = pat[copy_idx % len(pat)]
                        if eng == "s":
                            nc.scalar.activation(
                                dst,
                                src,
                                mybir.ActivationFunctionType.Identity,
                                bias=tt[:pxw, col : col + 1],
                            )
                        else:
                            nc.vector.tensor_scalar_add(
                                dst, src, tt[:pxw, col : col + 1]
                            )
                        copy_idx += 1
                offA = g * PA * group_w
                offB = PA * N_LOC + g * PB * group_w
                for (off, pxw, gt_t) in ((offA, PA, gtA), (offB, PB, gtB)):
                    dest = out_flat[off : off + pxw * group_w].rearrange(
                        "(p c) -> p c", p=pxw
                    )
                    dma_engines[dma_idx % len(dma_engines)].dma_start(dest, gt_t[:])
                    dma_idx += 1

    nc.compile()
    return nc


def _build_nc_h2(cfg):
    """Row-tiled fp16 with single-copy evacuation (the consolidated layout).

    Like H (two concurrent row-group matmuls cover all 242 px in one
    8192-cycle sample pass) but:
      - K=18: tau enters via two const-one rows against (-tau) fp16
        hi/lo planes, so the PSUM->SBUF op is a plain dtype-cast copy;
      - the B chunk's stationary is zero-padded to 128 columns, so both
        matmuls write full 128-partition banks and ONE copy per slice
        evacuates A+B together (junk pixel rows ride along for free -
        partitions are parallel lanes on DVE/ACT);
      - pack=1 with 4 PSUM slots in flight decouples the MM/copy
        pipeline; copies alternate vector/scalar;
      - output ships [128, slices*2*512] fp16 blocks (incl ~8% junk
        rows); host strips them.
    """
    import concourse.mybir as mybir
    from concourse import bacc
    from concourse.tile import TileContext

    f32 = mybir.dt.float32
    f16 = mybir.dt.float16

    K = K_F16  # 18
    sw = cfg["slice_w"]
    group_w = cfg["group_w"]
    spg = group_w // sw
    n_slices = N_LOC // sw
    n_groups = N_LOC // group_w
    WPAD = 128

    nc = bacc.Bacc("TRN2", target_bir_lowering=False, debug=False)
    TileContext = _tc_class(TileContext, cfg)
    xw = nc.declare_dram_parameter("xw", [50, WPAD + N_LOC], f16, isOutput=False)
    out_flat = nc.declare_dram_parameter(
        "out_flat", [TILE * 2 * N_LOC], f16, isOutput=True
    )

    chunks = cfg["h_chunks"]

    with TileContext(nc) as tc:
        with (
            tc.tile_pool(name="xin", bufs=1) as xpool,
            tc.tile_pool(name="ps", bufs=cfg["psum_bufs"], space="PSUM") as pspool,
            tc.tile_pool(name="ob", bufs=1) as opool,
        ):
            in_eng = getattr(nc, cfg["in_eng"])
            in_eng2 = getattr(nc, cfg.get("in_eng2", cfg["in_eng"]))
            xt0 = xpool.tile([32 + K, WPAD + N_LOC], f16)
            if cfg.get("dma_warm"):
                # the first dma_start on an engine pays a ~0.3-0.9us
                # first-issue penalty; absorb it with a tiny transfer so
                # the real chunk0 issues at steady-state cost
                dwm1 = xpool.tile([1, 32], f16, tag="dwm1")
                dwm2 = xpool.tile([1, 32], f16, tag="dwm2")
                in_eng.dma_start(dwm1[:], xw[0:1, 0:32])
                in_eng2.dma_start(dwm2[:], xw[0:1, 0:32])
            lo = WPAD
            first = True
            for w in chunks:
                for rg in (0, 32):
                    a, b = (0, lo + w) if first else (lo, lo + w)
                    # first chunk pair goes out on two engines in parallel
                    # so the opening matmul pair starts ~0.7us earlier
                    eng = in_eng2 if (first and rg == 32) else in_eng
                    eng.dma_start(xt0[rg : rg + K, a:b], xw[rg : rg + K, a:b])
                first = False
                lo += w

            if cfg.get("warm_tables"):
                dw = xpool.tile([16, 64], f16, tag="warm")
                nc.vector.memset(dw[:], 0.0)
                dscr = xpool.tile([16, 32], f16, tag="wscr")
                nc.scalar.copy(dscr[:], dw[:, :32])  # pulls ACT_TABLE_LOAD early

            wA = xt0[0:K, 0:WPAD]
            wB = xt0[32 : 32 + K, 0:WPAD]
            copy_engines = {"s": nc.scalar.copy, "v": nc.vector.tensor_copy}
            dma_engines = [getattr(nc, e) for e in cfg["dma_engines"]]
            pat = cfg["copy_pattern"]

            # group widths in samples; a smaller final group shrinks the
            # fully-exposed last DMA
            gws = list(cfg.get("h_groups") or [group_w] * n_groups)
            assert sum(gws) == N_LOC and all(w % sw == 0 for w in gws)

            s_base = 0
            off = 0
            for g, gw in enumerate(gws):
                spg = gw // sw
                gt = opool.tile([TILE, spg * 2 * sw], f16, tag=f"g{g}")
                for sj in range(spg):
                    sl = s_base // sw + sj
                    s = WPAD + sl * sw
                    ps = pspool.tile([TILE, 2 * sw], f32)
                    nc.tensor.matmul(
                        ps[:, 0:sw],
                        wA,
                        xt0[0:K, s : s + sw],
                        start=True,
                        stop=True,
                        tile_position=(0, 0),
                    )
                    nc.tensor.matmul(
                        ps[:, sw : 2 * sw],
                        wB,
                        xt0[32 : 32 + K, s : s + sw],
                        start=True,
                        stop=True,
                        tile_position=(32, 0),
                    )
                    if cfg.get("split_last_copy") and sl == N_LOC // sw - 1:
                        # the very last copy gates the drain; run its two
                        # halves on both engines in parallel (~0.5us sooner)
                        nc.scalar.copy(
                            gt[:, sj * 2 * sw : sj * 2 * sw + sw], ps[:, 0:sw]
                        )
                        nc.vector.tensor_copy(
                            gt[:, sj * 2 * sw + sw : (sj + 1) * 2 * sw],
                            ps[:, sw : 2 * sw],
                        )
                    else:
                        copy_engines[pat[sl % len(pat)]](
                            gt[:, sj * 2 * sw : (sj + 1) * 2 * sw], ps[:]
                        )
                s_base += gw
                blk_w = spg * 2 * sw
                dest = out_flat[off : off + TILE * blk_w].rearrange(
                    "(p c) -> p c", p=TILE
                )
                dma_engines[g % len(dma_engines)].dma_start(dest, gt[:])
                off += TILE * blk_w

    # LDWEIGHTS dedup: the two stationaries (A at PE rows 0-17, B at rows
    # 32-49) never change and occupy disjoint row groups, so after each is
    # loaded once every further reload the Tile splitter emitted is
    # redundant. Shrink repeats to a single (idempotent) column: LDW cost
    # scales with column count, so they become ~free and the PE pull-ahead
    # hides them entirely.
    if cfg.get("ldw_dedup"):
        seen = set()
        for fn in nc.m.functions:
            for blk in fn.blocks:
                for ins in blk.instructions:
                    if type(ins).__name__ != "InstLdweights":
                        continue
                    ap = ins.ins[0]
                    key = (str(ins.tile_position), ap.concise(), ap.offset)
                    if key in seen:
                        l = ap.ap
                        if len(l) >= 2 and l[-1][1] > 1:
                            ap.ap = list(l[:-1]) + [[l[-1][0], 1]]
                    else:
                        seen.add(key)

    nc.compile()
    return nc


def _pack_host_h2(S_re, S_im, D_re, D_im, tau):
    X16, W16 = _features16(S_re, S_im, D_re, D_im, tau)
    Xh = X16.astype(np.float16)
    Wh = W16.astype(np.float16)
    ntau = -np.asarray(tau, dtype=np.float32)
    th = ntau.astype(np.float16)
    tl = (ntau - th.astype(np.float32)).astype(np.float16)
    PA, PB = PX_CHUNKS
    WPAD = 128

    in_maps = []
    for i in range(N_CORES):
        cols = np.zeros((50, WPAD + N_LOC), dtype=np.float16)
        # group A: px 0-127
        cols[0:16, :PA] = Wh[:, :PA]
        cols[16, :PA] = th[:PA]
        cols[17, :PA] = tl[:PA]
        # group B: px 128-241 (cols 114-127 stay zero)
        cols[32:48, :PB] = Wh[:, PA:]
        cols[48, :PB] = th[PA:]
        cols[49, :PB] = tl[PA:]
        xc = Xh[:, i * N_LOC : (i + 1) * N_LOC]
        cols[0:16, WPAD:] = xc
        cols[16:18, WPAD:] = np.float16(1.0)
        cols[32:48, WPAD:] = xc
        cols[48:50, WPAD:] = np.float16(1.0)
        in_maps.append({"xw": cols})
    return in_maps


def _unpack_host_h2(res, cfg):
    sw = cfg["slice_w"]
    PA, PB = PX_CHUNKS
    gws = list(cfg.get("h_groups") or [cfg["group_w"]] * (N_LOC // cfg["group_w"]))
    out = np.empty((N_SAMPLES, N_PX), dtype=np.float32)
    outT = np.empty((N_PX, N_LOC), dtype=np.float16)
    for i in range(N_CORES):
        buf = np.asarray(res.results[i]["out_flat"])
        # device layout: groups are contiguous, each [128, spg, 2, sw]
        off = 0
        s0 = 0
        for gw in gws:
            spg = gw // sw
            arr = buf[off : off + TILE * spg * 2 * sw].reshape(TILE, spg, 2, sw)
            outT[:PA, s0 : s0 + gw] = arr[:, :, 0, :].reshape(TILE, gw)
            outT[PA:, s0 : s0 + gw] = arr[:PB, :, 1, :].reshape(PB, gw)
            off += TILE * spg * 2 * sw
            s0 += gw
        out[i * N_LOC : (i + 1) * N_LOC] = outT.T.astype(np.float32)
    return out


def _pack_host_h(S_re, S_im, D_re, D_im, tau):
    X16, W16 = _features16(S_re, S_im, D_re, D_im, tau)
    Xh = X16.astype(np.float16)  # (16, N)
    Wh = W16.astype(np.float16)  # (16, 242)
    ntau = -np.asarray(tau, dtype=np.float32)
    PA, PB = PX_CHUNKS
    WPAD = 128

    ts_ = np.zeros((TILE, 2), dtype=np.float32)
    ts_[:PA, 0] = ntau[:PA]
    ts_[:PB, 1] = ntau[PA:]

    in_maps = []
    for i in range(N_CORES):
        cols = np.zeros((48, WPAD + N_LOC), dtype=np.float16)
        cols[0:16, :PA] = Wh[:, :PA]
        cols[32:48, :PB] = Wh[:, PA:]
        xc = Xh[:, i * N_LOC : (i + 1) * N_LOC]
        cols[0:16, WPAD:] = xc
        cols[32:48, WPAD:] = xc
        in_maps.append({"xw": cols, "taus": ts_})
    return in_maps


def _unpack_host_h(res, cfg):
    group_w = cfg["group_w"]
    n_groups = N_LOC // group_w
    PA, PB = PX_CHUNKS
    out = np.empty((N_SAMPLES, N_PX), dtype=np.float32)
    outT = np.empty((N_PX, N_LOC), dtype=np.float16)
    for i in range(N_CORES):
        buf = np.asarray(res.results[i]["out_flat"])
        a = buf[: PA * N_LOC].reshape(n_groups, PA, group_w)
        b = buf[PA * N_LOC :].reshape(n_groups, PB, group_w)
        for g in range(n_groups):
            outT[:PA, g * group_w : (g + 1) * group_w] = a[g]
            outT[PA:, g * group_w : (g + 1) * group_w] = b[g]
        out[i * N_LOC : (i + 1) * N_LOC] = outT.T.astype(np.float32)
    return out


def _unpack_host_g(res, cfg):
    group_w = cfg["group_w"]
    n_groups = N_LOC // group_w
    out = np.empty((N_SAMPLES, N_PX), dtype=np.float32)
    outT = np.empty((N_PX, N_LOC), dtype=np.float16)
    for i in range(N_CORES):
        buf = np.asarray(res.results[i]["out_flat"])
        off = 0
        px_lo = 0
        for px_w in PX_CHUNKS:
            for g in range(n_groups):
                blk = buf[off : off + px_w * group_w].reshape(px_w, group_w)
                outT[px_lo : px_lo + px_w, g * group_w : (g + 1) * group_w] = blk
                off += px_w * group_w
            px_lo += px_w
        out[i * N_LOC : (i + 1) * N_LOC] = outT.T.astype(np.float32)
    return out


def _build_nc_b(cfg):
    """Pixels-on-partitions layout with exact-fp32 bf16 3-way split.

    The bilinear form is reduced to 16 features per sample via the
    Hermitian symmetry of S (4 diag + 6 sym-offdiag + 6 antisym-offdiag,
    off-diagonal weights doubled). X16 = Xh+Xm+Xl, W16 = Wh+Wm+Wl (bf16
    planes); the six significant cross-terms (hh, mh, hm, lh, hl, mm) are
    K-stacked into ONE bf16 matmul with K = 6*16 = 96:
      [Xh;Xm;Xh;Xl;Xh;Xm] x [Wh;Wh;Wm;Wh;Wl;Wm]
    (dropped ml/lm/ll terms are ~2^-27 relative -> fp32-grade accuracy).
    Stationary operand is the W side (reloaded only on pixel-chunk switch),
    moving is samples, so the PE streams at 1 col/cycle. The output lands
    transposed (242 x 8192) and is unscrambled on the host. tau is applied
    per-partition during the PSUM->SBUF copy (ACT Identity bias / DVE
    tensor_scalar add). Pixel chunks are 128 + 114 so output DMAs engage
    all 16 SDMA engines (8 partitions each).
    """
    import concourse.mybir as mybir
    from concourse import bacc
    from concourse.tile import TileContext

    f32 = mybir.dt.float32
    bf16 = mybir.dt.bfloat16

    slice_w = cfg["slice_w"]
    group_w = cfg["group_w"]
    slices_per_group = group_w // slice_w
    n_groups = N_LOC // group_w

    nc = bacc.Bacc("TRN2", target_bir_lowering=False, debug=False)
    TileContext = _tc_class(TileContext, cfg)
    xw = nc.declare_dram_parameter("xw", [96, W_PAD + N_LOC], bf16, isOutput=False)
    taus = nc.declare_dram_parameter("taus", [128, 2], f32, isOutput=False)
    if cfg["linear_out"]:
        # each (px_w, group_w) staging tile lands as one contiguous HBM
        # block -> the DMA splits evenly across all 16 SDMA engines
        out_flat = nc.declare_dram_parameter("out_flat", [N_PX * N_LOC], f32, isOutput=True)
    else:
        outT = nc.declare_dram_parameter("outT", [N_PX, N_LOC], f32, isOutput=True)

    n_xchunks = cfg["x_chunks"]
    x_chunk = N_LOC // n_xchunks

    with TileContext(nc) as tc:
        with (
            tc.tile_pool(name="xin", bufs=1) as xpool,
            tc.tile_pool(name="ps", bufs=cfg["psum_bufs"], space="PSUM") as pspool,
            tc.tile_pool(name="ob", bufs=1) as opool,
        ):
            # W planes + tau first (small, fast), then per-chunk sample
            # tiles so early matmuls only wait on their own chunk's DMA
            in_eng = getattr(nc, cfg["in_eng"])
            wtile = xpool.tile([96, W_PAD], bf16, tag="w")
            in_eng.dma_start(wtile[:], xw[:, :W_PAD])
            tt = xpool.tile([128, 2], f32, tag="taus")
            in_eng.dma_start(tt[:], taus[:])
            xts = []
            for ci in range(n_xchunks):
                xt = xpool.tile([96, x_chunk], bf16, tag=f"x{ci}")
                lo = W_PAD + ci * x_chunk
                in_eng.dma_start(xt[:], xw[:, lo : lo + x_chunk])
                xts.append(xt)

            dma_engines = [getattr(nc, e) for e in cfg["dma_engines"]]
            copy_idx = 0
            dma_idx = 0
            px_lo = 0
            for c, px_w in enumerate(PX_CHUNKS):
                wa = wtile[:, px_lo : px_lo + px_w]
                for g in range(n_groups):
                    gt = opool.tile([px_w, group_w], f32, tag=f"g{c}_{g}")
                    for sj in range(slices_per_group):
                        s = g * slices_per_group + sj
                        ci, off = divmod(s * slice_w, x_chunk)
                        xm = xts[ci][:, off : off + slice_w]
                        ps = pspool.tile([px_w, slice_w], f32, tag="ps")
                        nc.tensor.matmul(ps[:], wa, xm, start=True, stop=True)
                        dst = gt[:, sj * slice_w : (sj + 1) * slice_w]
                        pat = cfg["copy_pattern"]
                        eng = pat[copy_idx % len(pat)]
                        if eng == "s":
                            nc.scalar.activation(
                                dst,
                                ps[:],
                                mybir.ActivationFunctionType.Identity,
                                bias=tt[:px_w, c : c + 1],
                            )
                        else:
                            nc.vector.tensor_scalar_add(
                                dst, ps[:], tt[:px_w, c : c + 1]
                            )
                        copy_idx += 1
                    if cfg["linear_out"]:
                        off = (px_lo * N_LOC) + g * px_w * group_w
                        dest = out_flat[off : off + px_w * group_w].rearrange(
                            "(p c) -> p c", p=px_w
                        )
                    else:
                        dest = outT[
                            px_lo : px_lo + px_w, g * group_w : (g + 1) * group_w
                        ]
                    dma_engines[dma_idx % len(dma_engines)].dma_start(dest, gt[:])
                    dma_idx += 1
                px_lo += px_w

    nc.compile()
    return nc


def _get_nc(cfg=None):
    cfg = dict(DEFAULT_CFG, **(cfg or {}))
    key = tuple(sorted((k, str(v)) for k, v in cfg.items()))
    if key not in _BUILT:
        builder = {
            "B": _build_nc_b,
            "F": _build_nc_f16,
            "G": _build_nc_g,
            "H": _build_nc_h,
            "H2": _build_nc_h2,
        }.get(cfg["layout"], _build_nc)
        _BUILT[key] = (builder(cfg), cfg)
    return _BUILT[key]


def _pack_host(S_re, S_im, D_re, D_im, tau, pxp):
    """Build per-core input maps: weights + transposed feature matrix."""
    Dr = np.asarray(D_re, dtype=np.float32)
    Di = np.asarray(D_im, dtype=np.float32)
    tau = np.asarray(tau, dtype=np.float32)

    Wr = Dr[:, None, :] * Dr[None, :, :] + Di[:, None, :] * Di[None, :, :]
    Wi = Di[:, None, :] * Dr[None, :, :] - Dr[:, None, :] * Di[None, :, :]
    W = np.empty((K_FEAT, pxp), dtype=np.float32)
    W[:, N_PX:] = 0.0
    W[:16, :N_PX] = Wr.reshape(16, N_PX)
    W[16:32, :N_PX] = Wi.reshape(16, N_PX)
    W[32, :N_PX] = -tau

    X = np.empty((K_FEAT, N_SAMPLES), dtype=np.float32)
    X[:16] = np.asarray(S_re, dtype=np.float32).reshape(N_SAMPLES, 16).T
    X[16:32] = np.asarray(S_im, dtype=np.float32).reshape(N_SAMPLES, 16).T
    X[32] = 1.0

    in_maps = []
    for i in range(N_CORES):
        xtw = np.empty((K_FEAT, pxp + N_LOC), dtype=np.float32)
        xtw[:, :pxp] = W
        xtw[:, pxp:] = X[:, i * N_LOC : (i + 1) * N_LOC]
        in_maps.append({"xTw": xtw})
    return in_maps


def _features(S_re, S_im, D_re, D_im, tau):
    """(32, N) feature matrix X32 and (32, 242) weight matrix W32 (fp32)."""
    Dr = np.asarray(D_re, dtype=np.float32)
    Di = np.asarray(D_im, dtype=np.float32)
    Wr = Dr[:, None, :] * Dr[None, :, :] + Di[:, None, :] * Di[None, :, :]
    Wi = Di[:, None, :] * Dr[None, :, :] - Dr[:, None, :] * Di[None, :, :]
    W32 = np.concatenate([Wr.reshape(16, N_PX), Wi.reshape(16, N_PX)], 0)
    X32 = np.empty((32, N_SAMPLES), dtype=np.float32)
    X32[:16] = np.asarray(S_re, dtype=np.float32).reshape(N_SAMPLES, 16).T
    X32[16:] = np.asarray(S_im, dtype=np.float32).reshape(N_SAMPLES, 16).T
    return X32, W32


def _bf16_planes(a):
    import ml_dtypes

    bf = ml_dtypes.bfloat16
    h = a.astype(bf)
    r = a - h.astype(np.float32)
    m = r.astype(bf)
    l = (r - m.astype(np.float32)).astype(bf)
    return h, m, l


def _features16(S_re, S_im, D_re, D_im, tau):
    """(16, N) reduced features and (16, 242) weights using Hermitian
    symmetry: 4 diagonal + 6 sym-offdiag (weight doubled) + 6 antisym-
    offdiag (weight doubled)."""
    Dr = np.asarray(D_re, dtype=np.float32)
    Di = np.asarray(D_im, dtype=np.float32)
    Wr = (Dr[:, None, :] * Dr[None, :, :] + Di[:, None, :] * Di[None, :, :]).reshape(
        16, N_PX
    )
    Wi = (Di[:, None, :] * Dr[None, :, :] - Dr[:, None, :] * Di[None, :, :]).reshape(
        16, N_PX
    )
    W16 = np.concatenate([Wr[IDX_DIAG], 2.0 * Wr[IDX_OFF], 2.0 * Wi[IDX_OFF]], 0)

    Sr = np.asarray(S_re, dtype=np.float32).reshape(N_SAMPLES, 16)
    Si = np.asarray(S_im, dtype=np.float32).reshape(N_SAMPLES, 16)
    X16 = np.empty((16, N_SAMPLES), dtype=np.float32)
    X16[0:4] = Sr[:, IDX_DIAG].T
    X16[4:10] = Sr[:, IDX_OFF].T
    X16[10:16] = Si[:, IDX_OFF].T
    return X16, W16


def _pack_host_f16(S_re, S_im, D_re, D_im, tau, tau_bias=False):
    X16, W16 = _features16(S_re, S_im, D_re, D_im, tau)
    Xh = X16.astype(np.float16)  # (16, N)
    Wh = W16.astype(np.float16)  # (16, 242)
    ntau = -np.asarray(tau, dtype=np.float32)

    if tau_bias:
        # tau applied as a per-partition bias during the PSUM->SBUF op
        ts_ = np.zeros((TILE, 2), dtype=np.float32)
        ts_[: PX_CHUNKS[0], 0] = ntau[: PX_CHUNKS[0]]
        ts_[: PX_CHUNKS[1], 1] = ntau[PX_CHUNKS[0] :]
        in_maps = []
        for i in range(N_CORES):
            cols = np.empty((16, N_PX + N_LOC), dtype=np.float16)
            cols[:, :N_PX] = Wh
            cols[:, N_PX:] = Xh[:, i * N_LOC : (i + 1) * N_LOC]
            in_maps.append({"xw": cols, "taus": ts_})
        return in_maps

    th = ntau.astype(np.float16)
    tl = (ntau - th.astype(np.float32)).astype(np.float16)

    in_maps = []
    for i in range(N_CORES):
        cols = np.empty((K_F16, N_PX + N_LOC), dtype=np.float16)
        cols[:16, :N_PX] = Wh
        cols[16, :N_PX] = th
        cols[17, :N_PX] = tl
        cols[:16, N_PX:] = Xh[:, i * N_LOC : (i + 1) * N_LOC]
        cols[16:, N_PX:] = np.float16(1.0)
        in_maps.append({"xw": cols})
    return in_maps


def _unpack_host_f16(res, groups):
    tpg = N_TILES // groups
    out = np.empty((N_SAMPLES, N_PX), dtype=np.float32)
    for i in range(N_CORES):
        buf = res.results[i]["out_flat"]
        # block layout per group: (partition p, tile-in-group j, col c);
        # tile t = g*tpg + j holds samples t*128 + p
        arr = np.asarray(buf).reshape(groups, TILE, tpg, N_PX)
        out[i * N_LOC : (i + 1) * N_LOC] = (
            arr.transpose(0, 2, 1, 3).reshape(N_LOC, N_PX).astype(np.float32)
        )
    return out


def _pack_host_b(S_re, S_im, D_re, D_im, tau):
    import ml_dtypes

    bf = ml_dtypes.bfloat16
    X16, W16 = _features16(S_re, S_im, D_re, D_im, tau)
    Xh, Xm, Xl = _bf16_planes(X16)
    Wh, Wm, Wl = _bf16_planes(W16)

    # K-stacked pairs: hh, mh, hm, lh, hl, mm
    wA = np.concatenate([Wh, Wh, Wm, Wh, Wl, Wm], 0)  # (96, 242)
    xstack = np.concatenate([Xh, Xm, Xh, Xl, Xh, Xm], 0)  # (96, N)

    taus = np.zeros((128, 2), dtype=np.float32)
    tau = np.asarray(tau, dtype=np.float32)
    taus[: PX_CHUNKS[0], 0] = -tau[: PX_CHUNKS[0]]
    taus[: PX_CHUNKS[1], 1] = -tau[PX_CHUNKS[0] :]

    in_maps = []
    for i in range(N_CORES):
        cols = np.empty((96, W_PAD + N_LOC), dtype=bf)
        cols[:, :W_PAD] = wA
        cols[:, W_PAD:] = xstack[:, i * N_LOC : (i + 1) * N_LOC]
        in_maps.append({"xw": cols, "taus": taus})
    return in_maps


def _pack_host_a16(S_re, S_im, D_re, D_im, tau, pmajor=False):
    import ml_dtypes

    bf = ml_dtypes.bfloat16
    X16, W16 = _features16(S_re, S_im, D_re, D_im, tau)
    Xh, Xm, Xl = _bf16_planes(X16)
    Wh, Wm, Wl = _bf16_planes(W16)
    th, tm, tl = _bf16_planes(-np.asarray(tau, dtype=np.float32)[None, :])

    ones = np.ones((1, N_SAMPLES), dtype=np.float32).astype(bf)
    xstack = np.concatenate([Xh, Xm, Xh, Xl, Xh, Xm, ones, ones, ones], 0)  # (99, N)
    wstack = np.concatenate([Wh, Wh, Wm, Wh, Wl, Wm, th, tm, tl], 0)        # (99, 242)

    in_maps = []
    for i in range(N_CORES):
        xcore = xstack[:, i * N_LOC : (i + 1) * N_LOC]
        if pmajor:
            # device tile t partition p holds sample p*64 + t
            xcore = np.ascontiguousarray(
                xcore.reshape(99, TILE, N_TILES).swapaxes(1, 2).reshape(99, N_LOC)
            )
        cols = np.empty((99, N_PX + N_LOC), dtype=bf)
        cols[:, :N_PX] = wstack
        cols[:, N_PX:] = xcore
        in_maps.append({"xTw": cols})
    return in_maps


def _run(inputs, trace=False, cfg=None):
    from concourse.bass_utils import run_bass_kernel_spmd

    nc, full_cfg = _get_nc(cfg)
    if full_cfg["layout"] == "H2":
        in_maps = _pack_host_h2(**inputs)
        res = run_bass_kernel_spmd(nc, in_maps, list(range(N_CORES)), trace=trace)
        out = _unpack_host_h2(res, full_cfg)
        return out, res
    if full_cfg["layout"] == "H":
        in_maps = _pack_host_h(**inputs)
        res = run_bass_kernel_spmd(nc, in_maps, list(range(N_CORES)), trace=trace)
        out = _unpack_host_h(res, full_cfg)
        return out, res
    if full_cfg["layout"] == "G":
        in_maps = _pack_host_f16(**inputs, tau_bias=full_cfg.get("tau_bias", False))
        res = run_bass_kernel_spmd(nc, in_maps, list(range(N_CORES)), trace=trace)
        out = _unpack_host_g(res, full_cfg)
        return out, res
    if full_cfg["layout"] == "F":
        in_maps = _pack_host_f16(**inputs)
        res = run_bass_kernel_spmd(nc, in_maps, list(range(N_CORES)), trace=trace)
        out = _unpack_host_f16(res, full_cfg["groups"])
        return out, res
    if full_cfg["layout"] == "B":
        in_maps = _pack_host_b(**inputs)
        res = run_bass_kernel_spmd(nc, in_maps, list(range(N_CORES)), trace=trace)
        out = np.empty((N_SAMPLES, N_PX), dtype=np.float32)
        n_groups = N_LOC // full_cfg["group_w"]
        for i in range(N_CORES):
            if full_cfg["linear_out"]:
                buf = res.results[i]["out_flat"]
                outT = np.empty((N_PX, N_LOC), dtype=np.float32)
                off = 0
                px_lo = 0
                for px_w in PX_CHUNKS:
                    gw = full_cfg["group_w"]
                    for g in range(n_groups):
                        blk = buf[off : off + px_w * gw].reshape(px_w, gw)
                        outT[px_lo : px_lo + px_w, g * gw : (g + 1) * gw] = blk
                        off += px_w * gw
                    px_lo += px_w
            else:
                outT = res.results[i]["outT"]
            out[i * N_LOC : (i + 1) * N_LOC] = outT.T
    elif full_cfg.get("bf16split"):
        in_maps = _pack_host_a16(**inputs, pmajor=full_cfg.get("pmajor", False))
        res = run_bass_kernel_spmd(nc, in_maps, list(range(N_CORES)), trace=trace)
        out = np.concatenate(
            [res.results[i]["out"] for i in range(N_CORES)], axis=0
        )
        return out, res
    else:
        pxp = 256 if full_cfg["f32r"] else N_PX
        in_maps = _pack_host(**inputs, pxp=pxp)
        res = run_bass_kernel_spmd(nc, in_maps, list(range(N_CORES)), trace=trace)
        out = np.concatenate(
            [res.results[i]["out"] for i in range(N_CORES)], axis=0
        )
    return out, res


def kernel(**inputs) -> np.ndarray:
    out, _ = _run(inputs, trace=False)
    return out

